# revision 1
# baseline (speedup 1.0000x reference)
"""Trainium2 Bass kernel for nn_Experts (grouped MoE expert MLP).

Computes, for each of 8 experts e:
    h   = x_e @ w0_e.T          # [2048,1024] @ [1024,4096] -> [2048,4096]
    g   = gelu_exact(h)
    out = g @ w3_e.T            # [2048,4096] @ [4096,1024] -> [2048,1024]
then masks unpopular experts with zero gating activity (output_tensor).

Sharding: expert-parallel, 1 expert per NeuronCore across 8 cores (SPMD —
one compiled NEFF, per-core input data).

Layout strategy: all operands are pre-transposed on the host into
contraction-major ("K-major") layouts so the device kernel needs no
transposes at all:
    xT  [128, 8, 2048]  (d%128, d//128, t)   bf16
    w0T [128, 8, 4096]  (d%128, d//128, f)   bf16
    w3T [128, 32, 1024] (f%128, f//128, d)   bf16
GEMM1 produces hT tiles [f=128, t] in PSUM, GELU moves them to SBUF as bf16,
and those tiles are directly the lhsT operand of GEMM2 (contraction over f),
whose PSUM output [t=128, d] accumulates over all 32 f-chunks and lands in
the natural [t, d] layout of the output.
"""

import numpy as np
import ml_dtypes

T = 2048      # tokens (capacity) per expert
D = 1024      # hidden
F = 4096      # ffn
P = 128       # partitions
TB = 256      # token block (GEMM1 moving free dim)
NTB = T // TB
DC = D // P   # 8 d-chunks (GEMM1 contraction)
FC = F // P   # 32 f-chunks (GEMM2 contraction)
DW = 512      # GEMM2 output free-dim chunk
NUM_LOCAL = 4
N_CORES = 8

_cache = {}


def _build_nc(
    tb_size=TB,          # token block
    x_split=1,           # extra splits of each x d-chunk DMA (along t)
    w0_split=1,          # extra splits of each w0 d-chunk DMA (along f)
    w3_group=1,          # f-chunks per w3 DMA
    g_bufs=4,
    h_bufs=2,
    o_sb_bufs=4,
    dma_scheme="tuned",  # "simple" | "tuned" (critical-prefix-first ordering)
    fcg=4,               # fc per w0/w3 DMA group in tuned scheme
    pipeline_o=True,     # issue GEMM2(fc) after GEMM1(fc+1) to hide gelu latency
):
    import sys
    if "/opt/trn_rl_repo" not in sys.path:
        sys.path.insert(0, "/opt/trn_rl_repo")
    import concourse.bass as bass
    import concourse.tile as tile
    import concourse.mybir as mybir
    from concourse import bacc

    bf16 = mybir.dt.bfloat16
    f32 = mybir.dt.float32
    AFT = mybir.ActivationFunctionType

    TBS = tb_size
    NTBS = T // TBS
    NTS = TBS // P       # t-subchunks per block (GEMM2 lhsT count)
    n_ops = NTS * 2      # out psum tiles per block ([t 128] x [d 512])

    nc = bacc.Bacc(
        "TRN2",
        target_bir_lowering=False,
        debug=False,
        enable_asserts=True,
        num_devices=N_CORES,
    )

    xT = nc.dram_tensor("xT", [P, DC, T], bf16, kind="ExternalInput").ap()
    w0T = nc.dram_tensor("w0T", [P, DC, F], bf16, kind="ExternalInput").ap()
    w3T = nc.dram_tensor("w3T", [P, FC, D], bf16, kind="ExternalInput").ap()
    out = nc.dram_tensor("out", [T, D], f32, kind="ExternalOutput").ap()

    with tile.TileContext(nc) as tc:
        with (
            tc.tile_pool(name="weights", bufs=1) as wpool,
            tc.tile_pool(name="gelu", bufs=g_bufs) as gpool,
            tc.tile_pool(name="ostage", bufs=o_sb_bufs) as opool,
            tc.tile_pool(name="hps", bufs=h_bufs, space="PSUM") as hpsum,
            tc.tile_pool(name="ops", bufs=n_ops, space="PSUM") as opsum,
        ):
            x_sb = wpool.tile([P, DC, T], bf16, name="x_sb", tag="x_sb")
            w0_sb = wpool.tile([P, DC, F], bf16, name="w0_sb", tag="w0_sb")
            w3_sb = wpool.tile([P, FC, D], bf16, name="w3_sb", tag="w3_sb")

            if dma_scheme == "simple":
                # Load x and w0 first (first h-tile needs ALL d-chunks of
                # both); w3 f-chunks stream in behind.
                for dc in range(DC):
                    for s in range(x_split):
                        w = T // x_split
                        nc.sync.dma_start(x_sb[:, dc, s * w:(s + 1) * w],
                                          xT[:, dc, s * w:(s + 1) * w])
                    for s in range(w0_split):
                        w = F // w0_split
                        nc.sync.dma_start(w0_sb[:, dc, s * w:(s + 1) * w],
                                          w0T[:, dc, s * w:(s + 1) * w])
                for g in range(FC // w3_group):
                    lo, hi = g * w3_group, (g + 1) * w3_group
                    nc.sync.dma_start(w3_sb[:, lo:hi], w3T[:, lo:hi])
            else:
                # Critical-prefix-first: x for tb0, then per-f-group w0 (all
                # d-chunks) and w3 interleaved in the order GEMM1/GEMM2
                # consume them, then the rest of x.
                for dc in range(DC):
                    nc.sync.dma_start(x_sb[:, dc, 0:TBS], xT[:, dc, 0:TBS])
                for g in range(FC // fcg):
                    flo, fhi = g * fcg * P, (g + 1) * fcg * P
                    for dc in range(DC):
                        nc.sync.dma_start(w0_sb[:, dc, flo:fhi],
                                          w0T[:, dc, flo:fhi])
                    nc.sync.dma_start(w3_sb[:, g * fcg:(g + 1) * fcg],
                                      w3T[:, g * fcg:(g + 1) * fcg])
                for tb in range(1, NTBS):
                    for dc in range(DC):
                        nc.sync.dma_start(
                            x_sb[:, dc, tb * TBS:(tb + 1) * TBS],
                            xT[:, dc, tb * TBS:(tb + 1) * TBS])

            for tb in range(NTBS):
                o_ps = [
                    opsum.tile([P, DW], f32, name=f"o_ps_{tb}_{i}", tag="o_ps")
                    for i in range(n_ops)
                ]

                def emit_o(fc, g_sb):
                    for ts in range(NTS):
                        for dc2 in range(2):
                            nc.tensor.matmul(
                                o_ps[ts * 2 + dc2][:],
                                g_sb[:, ts * P:(ts + 1) * P],
                                w3_sb[:, fc, dc2 * DW:(dc2 + 1) * DW],
                                start=(fc == 0),
                                stop=(fc == FC - 1),
                            )

                pending = None
                for fc in range(FC):
                    h_ps = hpsum.tile([P, TBS], f32, name=f"h_ps_{tb}_{fc}", tag="h_ps")
                    for dc in range(DC):
                        nc.tensor.matmul(
                            h_ps[:],
                            w0_sb[:, dc, fc * P:(fc + 1) * P],
                            x_sb[:, dc, tb * TBS:(tb + 1) * TBS],
                            start=(dc == 0),
                            stop=(dc == DC - 1),
                        )
                    g_sb = gpool.tile([P, TBS], bf16, name=f"g_{tb}_{fc}", tag="g")
                    nc.scalar.activation(g_sb[:], h_ps[:], AFT.Gelu)
                    if not pipeline_o:
                        emit_o(fc, g_sb)
                    else:
                        if pending is not None:
                            emit_o(*pending)
                        pending = (fc, g_sb)
                if pending is not None:
                    emit_o(*pending)

                for ts in range(NTS):
                    for dc2 in range(2):
                        o_sb = opool.tile([P, DW], f32, name=f"o_sb_{tb}_{ts}_{dc2}",
                                          tag="o_sb")
                        nc.vector.tensor_copy(o_sb[:], o_ps[ts * 2 + dc2][:])
                        nc.sync.dma_start(
                            out[tb * TBS + ts * P: tb * TBS + (ts + 1) * P,
                                dc2 * DW:(dc2 + 1) * DW],
                            o_sb[:],
                        )

    nc.compile()
    return nc


def _build_nc_v2(
    g_extra=0,           # extra gelu-tile slots beyond FC (lookahead into next block)
    h_bufs=3,
    o_ps_bufs=2,
    o_sb_bufs=3,
    x_bufs=2,
    fcg=4,               # fc per w0/w3 DMA group
    x_coarse=True,       # one DMA per x block vs per-dc
    w0_coarse=False,     # one DMA per w0 f-group vs per-dc
    warmup_mms=8,        # scratch matmuls issued before the real work so the
                         # PE rides out the HAM cold-clock window during the
                         # initial DMA wait instead of during real matmuls
):
    """TB=512 two-phase variant: per 512-token block, phase A runs GEMM1+GELU
    for all 32 f-chunks (g tiles [128,512] bf16 stay in SBUF), phase B runs
    GEMM2 as 8 sequential PSUM accumulation groups (one [t=128, d=512] output
    tile each, contraction over all 32 f-chunks). x is streamed per-block
    instead of fully resident to stay under the SBUF cap."""
    import sys
    if "/opt/trn_rl_repo" not in sys.path:
        sys.path.insert(0, "/opt/trn_rl_repo")
    import concourse.tile as tile
    import concourse.mybir as mybir
    from concourse import bacc

    bf16 = mybir.dt.bfloat16
    f32 = mybir.dt.float32
    AFT = mybir.ActivationFunctionType

    TBS = 512
    NTBS = T // TBS      # 4
    NTS = TBS // P       # 4

    G = FC // fcg        # w0 DMA groups
    FW = fcg * P         # f elements per group (512)

    nc = bacc.Bacc(
        "TRN2",
        target_bir_lowering=False,
        debug=False,
        enable_asserts=True,
        num_devices=N_CORES,
    )

    # DRAM layouts are grouped so every load has long (8KB) contiguous
    # per-partition runs: xT by token-block, w0T by f-group.
    xT = nc.dram_tensor("xT", [P, NTBS, DC, TBS], bf16, kind="ExternalInput").ap()
    w0T = nc.dram_tensor("w0T", [P, G, DC, FW], bf16, kind="ExternalInput").ap()
    w3T = nc.dram_tensor("w3T", [P, FC, D], bf16, kind="ExternalInput").ap()
    out = nc.dram_tensor("out", [T, D], f32, kind="ExternalOutput").ap()

    with tile.TileContext(nc) as tc:
        with (
            tc.tile_pool(name="weights", bufs=1) as wpool,
            tc.tile_pool(name="xin", bufs=x_bufs) as xpool,
            tc.tile_pool(name="gelu", bufs=FC + g_extra) as gpool,
            tc.tile_pool(name="ostage", bufs=o_sb_bufs) as opool,
            tc.tile_pool(name="hps", bufs=h_bufs, space="PSUM") as hpsum,
            tc.tile_pool(name="ops", bufs=o_ps_bufs, space="PSUM") as opsum,
        ):
            # w0 SBUF mirrors the grouped DRAM layout; GEMM1 slices
            # [:, fc//fcg, dc, (fc%fcg)*P : +P].
            w0_sb = wpool.tile([P, G, DC, FW], bf16, name="w0_sb", tag="w0_sb")
            w3_sb = wpool.tile([P, FC, D], bf16, name="w3_sb", tag="w3_sb")

            x_tiles = {}
            def load_x(tb):
                xt = xpool.tile([P, DC, TBS], bf16, name=f"x_{tb}", tag="x")
                if x_coarse:
                    nc.sync.dma_start(xt[:], xT[:, tb])
                else:
                    for dc in range(DC):
                        nc.sync.dma_start(xt[:, dc], xT[:, tb, dc])
                x_tiles[tb] = xt

            if warmup_mms:
                with (
                    tc.tile_pool(name="warm", bufs=1) as warmpool,
                    tc.tile_pool(name="warmps", bufs=1, space="PSUM") as warmpsum,
                ):
                    wsrc = warmpool.tile([P, DW], bf16, name="wsrc", tag="wsrc")
                    wps = warmpsum.tile([P, DW], f32, name="wps", tag="wps")
                    nc.gpsimd.memset(wsrc[:], 0.0)
                    for i in range(warmup_mms):
                        nc.tensor.matmul(wps[:], wsrc[:, :P], wsrc[:],
                                         start=(i == 0), stop=(i == warmup_mms - 1))

            # critical prefix: x[tb0], then w0/w3 by f-group in consumption order
            load_x(0)
            for g in range(G):
                if w0_coarse:
                    nc.sync.dma_start(w0_sb[:, g], w0T[:, g])
                else:
                    for dc in range(DC):
                        nc.sync.dma_start(w0_sb[:, g, dc], w0T[:, g, dc])
                nc.sync.dma_start(w3_sb[:, g * fcg:(g + 1) * fcg],
                                  w3T[:, g * fcg:(g + 1) * fcg])

            for tb in range(NTBS):
                if tb + 1 < NTBS:
                    load_x(tb + 1)
                xt = x_tiles.pop(tb)
                # phase A: GEMM1 + GELU for all fc
                g_tiles = []
                for fc in range(FC):
                    h_ps = hpsum.tile([P, TBS], f32, name=f"h_{tb}_{fc}", tag="h_ps")
                    for dc in range(DC):
                        j = fc % fcg
                        nc.tensor.matmul(
                            h_ps[:],
                            w0_sb[:, fc // fcg, dc, j * P:(j + 1) * P],
                            xt[:, dc],
                            start=(dc == 0),
                            stop=(dc == DC - 1),
                        )
                    g_sb = gpool.tile([P, TBS], bf16, name=f"g_{tb}_{fc}", tag="g")
                    nc.scalar.activation(g_sb[:], h_ps[:], AFT.Gelu)
                    g_tiles.append(g_sb)
                # phase B: GEMM2, one [t=128, d=512] accumulation group at a time
                for ts in range(NTS):
                    for dc2 in range(2):
                        o_ps = opsum.tile([P, DW], f32, name=f"o_{tb}_{ts}_{dc2}",
                                          tag="o_ps")
                        for fc in range(FC):
                            nc.tensor.matmul(
                                o_ps[:],
                                g_tiles[fc][:, ts * P:(ts + 1) * P],
                                w3_sb[:, fc, dc2 * DW:(dc2 + 1) * DW],
                                start=(fc == 0),
                                stop=(fc == FC - 1),
                            )
                        o_sb = opool.tile([P, DW], f32, name=f"os_{tb}_{ts}_{dc2}",
                                          tag="o_sb")
                        nc.vector.tensor_copy(o_sb[:], o_ps[:])
                        nc.sync.dma_start(
                            out[tb * TBS + ts * P: tb * TBS + (ts + 1) * P,
                                dc2 * DW:(dc2 + 1) * DW],
                            o_sb[:],
                        )

    nc.compile()
    return nc


def _get_nc():
    # v1 (_build_nc) predates the grouped DRAM layouts and is kept only for
    # reference; the host prep below feeds _build_nc_v2's layouts.
    if "nc" not in _cache:
        _cache["nc"] = _build_nc_v2()
    return _cache["nc"]


def _make_cached_fn(nc):
    """Build a reusable jitted 8-core executable around bass2jax's bass_exec
    primitive (the same lowering run_bass_kernel_spmd uses under axon), so
    repeat kernel() calls skip retrace/relower."""
    import jax
    import numpy as np
    from jax.sharding import Mesh, PartitionSpec
    try:
        from jax.experimental.shard_map import shard_map
    except ImportError:
        from jax.shard_map import shard_map
    import concourse.mybir as mybir
    from concourse.bass2jax import (_bass_exec_p, install_neuronx_cc_hook,
                                    partition_id_tensor)

    install_neuronx_cc_hook()
    partition_name = nc.partition_id_tensor.name if nc.partition_id_tensor else None
    in_names, out_names, out_avals, zero_shapes = [], [], [], []
    for alloc in nc.m.functions[0].allocations:
        if not isinstance(alloc, mybir.MemoryLocationSet):
            continue
        name = alloc.memorylocations[0].name
        if alloc.kind == "ExternalInput":
            if name != partition_name:
                in_names.append(name)
        elif alloc.kind == "ExternalOutput":
            out_names.append(name)
            shape = tuple(alloc.tensor_shape)
            dtype = mybir.dt.np(alloc.dtype)
            out_avals.append(jax.core.ShapedArray(shape, dtype))
            zero_shapes.append((shape, dtype))
    n_params = len(in_names)
    all_in_names = list(in_names) + list(out_names)
    if partition_name is not None:
        all_in_names.append(partition_name)

    def _body(*args):
        ins = list(args[:n_params])
        outs = list(args[n_params:])
        extra = [partition_id_tensor()] if partition_name is not None else []
        return tuple(_bass_exec_p.bind(
            *ins, *outs, *extra,
            out_avals=tuple(out_avals),
            in_names=tuple(all_in_names),
            out_names=tuple(out_names),
            lowering_input_output_aliases=(),
            sim_require_finite=True,
            sim_require_nnan=True,
            nc=nc,
        ))

    devices = jax.devices()[:N_CORES]
    mesh = Mesh(np.asarray(devices), ("core",))
    fn = jax.jit(
        shard_map(_body, mesh=mesh,
                  in_specs=(PartitionSpec("core"),) * (n_params + len(out_names)),
                  out_specs=(PartitionSpec("core"),) * len(out_names),
                  check_rep=False),
        keep_unused=True)

    def run(in_maps):
        concat_in = [np.concatenate([np.asarray(m[n]) for m in in_maps], axis=0)
                     for n in in_names]
        concat_zeros = [np.zeros((N_CORES * s[0], *s[1:]), dt)
                        for s, dt in zero_shapes]
        outs = fn(*concat_in, *concat_zeros)
        return [
            {name: np.asarray(outs[i]).reshape(N_CORES, *out_avals[i].shape)[c]
             for i, name in enumerate(out_names)}
            for c in range(N_CORES)
        ]

    return run


def kernel(**inputs):
    import os
    import sys
    if "/opt/trn_rl_repo" not in sys.path:
        sys.path.insert(0, "/opt/trn_rl_repo")
    from concourse import bass_utils

    output_tensor = np.asarray(inputs["output_tensor"], dtype=np.float32)  # [1, 8]
    x = np.asarray(inputs["inputs"], dtype=np.float32)   # [1, 8, 2048, 1024]
    w0 = np.asarray(inputs["w0"], dtype=np.float32)      # [8, 4096, 1024]
    w3 = np.asarray(inputs["w3"], dtype=np.float32)      # [8, 1024, 4096]

    bf = ml_dtypes.bfloat16
    TBS, NTBS, FCG = 512, T // 512, 4
    G, FW = FC // FCG, FCG * P

    def prep_expert(e):
        # cast to bf16 first (halves bytes moved by the transposes)
        xe = x[0, e].astype(bf)     # [t, d]
        w0e = w0[e].astype(bf)      # [f, d]
        w3e = w3[e].astype(bf)      # [d, f]
        # Layouts are contraction-major (partition = contraction dim % 128)
        # and grouped by DMA unit so each load is one long contiguous run per
        # partition:
        #   xT  [128, 4 tb, 8 dc, 512 t],  w0T [128, 8 g, 8 dc, 512 f],
        #   w3T [128, 32 fc, 1024 d]
        return {
            "xT": np.ascontiguousarray(
                xe.T.reshape(DC, P, NTBS, TBS).transpose(1, 2, 0, 3)),
            "w0T": np.ascontiguousarray(
                w0e.T.reshape(DC, P, G, FW).transpose(1, 2, 0, 3)),
            "w3T": np.ascontiguousarray(
                w3e.T.reshape(FC, P, D).transpose(1, 0, 2)),
        }

    from concurrent.futures import ThreadPoolExecutor
    with ThreadPoolExecutor(max_workers=N_CORES) as pool:
        in_maps = list(pool.map(prep_expert, range(N_CORES)))

    nc = _get_nc()
    results = None
    if "fast_fn" in _cache:
        try:
            results = _cache["fast_fn"](in_maps)
        except Exception:
            results = None
    if results is None:
        try:
            results = bass_utils.run_bass_kernel_spmd(
                nc, in_maps, core_ids=list(range(N_CORES))).results
        except ModuleNotFoundError:
            # trace path requested via env but axon NTFF hook missing
            os.environ["BASS_NEVER_TRACE"] = "1"
            results = bass_utils.run_bass_kernel_spmd(
                nc, in_maps, core_ids=list(range(N_CORES))).results
        try:
            fast = _make_cached_fn(nc)
            fast(in_maps)  # warm: jit trace + XLA/NEFF compile happens here
            _cache["fast_fn"] = fast
        except Exception:
            pass
    out_full = np.stack([results[e]["out"] for e in range(N_CORES)])[None]

    # unpopular experts with zero gating activity produce zeros
    unpop = output_tensor[:, NUM_LOCAL:].sum(axis=0) != 0
    mask = np.concatenate([np.ones(NUM_LOCAL, dtype=bool), unpop])
    out_full = out_full * mask[None, :, None, None].astype(np.float32)
    return out_full.astype(np.float32)



# revision 4
# speedup vs baseline: 1.3109x; 1.3109x over previous
"""Trainium2 Bass kernel for nn_Experts (grouped MoE expert MLP).

Computes, for each of 8 experts e:
    h   = x_e @ w0_e.T          # [2048,1024] @ [1024,4096] -> [2048,4096]
    g   = gelu_exact(h)
    out = g @ w3_e.T            # [2048,4096] @ [4096,1024] -> [2048,1024]
then masks unpopular experts with zero gating activity (output_tensor).

Sharding: expert-parallel, 1 expert per NeuronCore across 8 cores (SPMD —
one compiled NEFF, per-core input data).

Layout strategy: all operands are pre-transposed on the host into
contraction-major ("K-major") layouts so the device kernel needs no
transposes at all:
    xT  [128, 8, 2048]  (d%128, d//128, t)   bf16
    w0T [128, 8, 4096]  (d%128, d//128, f)   bf16
    w3T [128, 32, 1024] (f%128, f//128, d)   bf16
GEMM1 produces hT tiles [f=128, t] in PSUM, GELU moves them to SBUF as bf16,
and those tiles are directly the lhsT operand of GEMM2 (contraction over f),
whose PSUM output [t=128, d] accumulates over all 32 f-chunks and lands in
the natural [t, d] layout of the output.
"""

import numpy as np
import ml_dtypes

T = 2048      # tokens (capacity) per expert
D = 1024      # hidden
F = 4096      # ffn
P = 128       # partitions
TB = 256      # token block (GEMM1 moving free dim)
NTB = T // TB
DC = D // P   # 8 d-chunks (GEMM1 contraction)
FC = F // P   # 32 f-chunks (GEMM2 contraction)
DW = 512      # GEMM2 output free-dim chunk
NUM_LOCAL = 4
N_CORES = 8

_cache = {}


def _build_nc(
    tb_size=TB,          # token block
    x_split=1,           # extra splits of each x d-chunk DMA (along t)
    w0_split=1,          # extra splits of each w0 d-chunk DMA (along f)
    w3_group=1,          # f-chunks per w3 DMA
    g_bufs=4,
    h_bufs=2,
    o_sb_bufs=4,
    dma_scheme="tuned",  # "simple" | "tuned" (critical-prefix-first ordering)
    fcg=4,               # fc per w0/w3 DMA group in tuned scheme
    pipeline_o=True,     # issue GEMM2(fc) after GEMM1(fc+1) to hide gelu latency
):
    import sys
    if "/opt/trn_rl_repo" not in sys.path:
        sys.path.insert(0, "/opt/trn_rl_repo")
    import concourse.bass as bass
    import concourse.tile as tile
    import concourse.mybir as mybir
    from concourse import bacc

    bf16 = mybir.dt.bfloat16
    f32 = mybir.dt.float32
    AFT = mybir.ActivationFunctionType

    TBS = tb_size
    NTBS = T // TBS
    NTS = TBS // P       # t-subchunks per block (GEMM2 lhsT count)
    n_ops = NTS * 2      # out psum tiles per block ([t 128] x [d 512])

    nc = bacc.Bacc(
        "TRN2",
        target_bir_lowering=False,
        debug=False,
        enable_asserts=True,
        num_devices=N_CORES,
    )

    xT = nc.dram_tensor("xT", [P, DC, T], bf16, kind="ExternalInput").ap()
    w0T = nc.dram_tensor("w0T", [P, DC, F], bf16, kind="ExternalInput").ap()
    w3T = nc.dram_tensor("w3T", [P, FC, D], bf16, kind="ExternalInput").ap()
    out = nc.dram_tensor("out", [T, D], f32, kind="ExternalOutput").ap()

    with tile.TileContext(nc) as tc:
        with (
            tc.tile_pool(name="weights", bufs=1) as wpool,
            tc.tile_pool(name="gelu", bufs=g_bufs) as gpool,
            tc.tile_pool(name="ostage", bufs=o_sb_bufs) as opool,
            tc.tile_pool(name="hps", bufs=h_bufs, space="PSUM") as hpsum,
            tc.tile_pool(name="ops", bufs=n_ops, space="PSUM") as opsum,
        ):
            x_sb = wpool.tile([P, DC, T], bf16, name="x_sb", tag="x_sb")
            w0_sb = wpool.tile([P, DC, F], bf16, name="w0_sb", tag="w0_sb")
            w3_sb = wpool.tile([P, FC, D], bf16, name="w3_sb", tag="w3_sb")

            if dma_scheme == "simple":
                # Load x and w0 first (first h-tile needs ALL d-chunks of
                # both); w3 f-chunks stream in behind.
                for dc in range(DC):
                    for s in range(x_split):
                        w = T // x_split
                        nc.sync.dma_start(x_sb[:, dc, s * w:(s + 1) * w],
                                          xT[:, dc, s * w:(s + 1) * w])
                    for s in range(w0_split):
                        w = F // w0_split
                        nc.sync.dma_start(w0_sb[:, dc, s * w:(s + 1) * w],
                                          w0T[:, dc, s * w:(s + 1) * w])
                for g in range(FC // w3_group):
                    lo, hi = g * w3_group, (g + 1) * w3_group
                    nc.sync.dma_start(w3_sb[:, lo:hi], w3T[:, lo:hi])
            else:
                # Critical-prefix-first: x for tb0, then per-f-group w0 (all
                # d-chunks) and w3 interleaved in the order GEMM1/GEMM2
                # consume them, then the rest of x.
                for dc in range(DC):
                    nc.sync.dma_start(x_sb[:, dc, 0:TBS], xT[:, dc, 0:TBS])
                for g in range(FC // fcg):
                    flo, fhi = g * fcg * P, (g + 1) * fcg * P
                    for dc in range(DC):
                        nc.sync.dma_start(w0_sb[:, dc, flo:fhi],
                                          w0T[:, dc, flo:fhi])
                    nc.sync.dma_start(w3_sb[:, g * fcg:(g + 1) * fcg],
                                      w3T[:, g * fcg:(g + 1) * fcg])
                for tb in range(1, NTBS):
                    for dc in range(DC):
                        nc.sync.dma_start(
                            x_sb[:, dc, tb * TBS:(tb + 1) * TBS],
                            xT[:, dc, tb * TBS:(tb + 1) * TBS])

            for tb in range(NTBS):
                o_ps = [
                    opsum.tile([P, DW], f32, name=f"o_ps_{tb}_{i}", tag="o_ps")
                    for i in range(n_ops)
                ]

                def emit_o(fc, g_sb):
                    for ts in range(NTS):
                        for dc2 in range(2):
                            nc.tensor.matmul(
                                o_ps[ts * 2 + dc2][:],
                                g_sb[:, ts * P:(ts + 1) * P],
                                w3_sb[:, fc, dc2 * DW:(dc2 + 1) * DW],
                                start=(fc == 0),
                                stop=(fc == FC - 1),
                            )

                pending = None
                for fc in range(FC):
                    h_ps = hpsum.tile([P, TBS], f32, name=f"h_ps_{tb}_{fc}", tag="h_ps")
                    for dc in range(DC):
                        nc.tensor.matmul(
                            h_ps[:],
                            w0_sb[:, dc, fc * P:(fc + 1) * P],
                            x_sb[:, dc, tb * TBS:(tb + 1) * TBS],
                            start=(dc == 0),
                            stop=(dc == DC - 1),
                        )
                    g_sb = gpool.tile([P, TBS], bf16, name=f"g_{tb}_{fc}", tag="g")
                    nc.scalar.activation(g_sb[:], h_ps[:], AFT.Gelu)
                    if not pipeline_o:
                        emit_o(fc, g_sb)
                    else:
                        if pending is not None:
                            emit_o(*pending)
                        pending = (fc, g_sb)
                if pending is not None:
                    emit_o(*pending)

                for ts in range(NTS):
                    for dc2 in range(2):
                        o_sb = opool.tile([P, DW], f32, name=f"o_sb_{tb}_{ts}_{dc2}",
                                          tag="o_sb")
                        nc.vector.tensor_copy(o_sb[:], o_ps[ts * 2 + dc2][:])
                        nc.sync.dma_start(
                            out[tb * TBS + ts * P: tb * TBS + (ts + 1) * P,
                                dc2 * DW:(dc2 + 1) * DW],
                            o_sb[:],
                        )

    nc.compile()
    return nc


def _build_nc_v2(
    g_extra=0,           # extra gelu-tile slots beyond FC (lookahead into next block)
    h_bufs=3,
    o_ps_bufs=2,
    o_sb_bufs=3,
    x_bufs=2,
    fcg=4,               # fc per w0/w3 DMA group
    x_coarse=True,       # one DMA per x block vs per-dc
    w0_coarse=False,     # one DMA per w0 f-group vs per-dc
    warmup_mms=8,        # scratch matmuls issued before the real work so the
                         # PE rides out the HAM cold-clock window during the
                         # initial DMA wait instead of during real matmuls
):
    """TB=512 two-phase variant: per 512-token block, phase A runs GEMM1+GELU
    for all 32 f-chunks (g tiles [128,512] bf16 stay in SBUF), phase B runs
    GEMM2 as 8 sequential PSUM accumulation groups (one [t=128, d=512] output
    tile each, contraction over all 32 f-chunks). x is streamed per-block
    instead of fully resident to stay under the SBUF cap."""
    import sys
    if "/opt/trn_rl_repo" not in sys.path:
        sys.path.insert(0, "/opt/trn_rl_repo")
    import concourse.tile as tile
    import concourse.mybir as mybir
    from concourse import bacc

    bf16 = mybir.dt.bfloat16
    f32 = mybir.dt.float32
    AFT = mybir.ActivationFunctionType

    TBS = 512
    NTBS = T // TBS      # 4
    NTS = TBS // P       # 4

    G = FC // fcg        # w0 DMA groups
    FW = fcg * P         # f elements per group (512)

    nc = bacc.Bacc(
        "TRN2",
        target_bir_lowering=False,
        debug=False,
        enable_asserts=True,
        num_devices=N_CORES,
    )

    # DRAM layouts are grouped so every load has long (8KB) contiguous
    # per-partition runs: xT by token-block, w0T by f-group.
    xT = nc.dram_tensor("xT", [P, NTBS, DC, TBS], bf16, kind="ExternalInput").ap()
    w0T = nc.dram_tensor("w0T", [P, G, DC, FW], bf16, kind="ExternalInput").ap()
    w3T = nc.dram_tensor("w3T", [P, FC, D], bf16, kind="ExternalInput").ap()
    out = nc.dram_tensor("out", [T, D], f32, kind="ExternalOutput").ap()

    with tile.TileContext(nc) as tc:
        with (
            tc.tile_pool(name="weights", bufs=1) as wpool,
            tc.tile_pool(name="xin", bufs=x_bufs) as xpool,
            tc.tile_pool(name="gelu", bufs=FC + g_extra) as gpool,
            tc.tile_pool(name="ostage", bufs=o_sb_bufs) as opool,
            tc.tile_pool(name="hps", bufs=h_bufs, space="PSUM") as hpsum,
            tc.tile_pool(name="ops", bufs=o_ps_bufs, space="PSUM") as opsum,
        ):
            # w0 SBUF mirrors the grouped DRAM layout; GEMM1 slices
            # [:, fc//fcg, dc, (fc%fcg)*P : +P].
            w0_sb = wpool.tile([P, G, DC, FW], bf16, name="w0_sb", tag="w0_sb")
            w3_sb = wpool.tile([P, FC, D], bf16, name="w3_sb", tag="w3_sb")

            x_tiles = {}
            def load_x(tb):
                xt = xpool.tile([P, DC, TBS], bf16, name=f"x_{tb}", tag="x")
                if x_coarse:
                    nc.sync.dma_start(xt[:], xT[:, tb])
                else:
                    for dc in range(DC):
                        nc.sync.dma_start(xt[:, dc], xT[:, tb, dc])
                x_tiles[tb] = xt

            if warmup_mms:
                with (
                    tc.tile_pool(name="warm", bufs=1) as warmpool,
                    tc.tile_pool(name="warmps", bufs=1, space="PSUM") as warmpsum,
                ):
                    wsrc = warmpool.tile([P, DW], bf16, name="wsrc", tag="wsrc")
                    wps = warmpsum.tile([P, DW], f32, name="wps", tag="wps")
                    nc.gpsimd.memset(wsrc[:], 0.0)
                    for i in range(warmup_mms):
                        nc.tensor.matmul(wps[:], wsrc[:, :P], wsrc[:],
                                         start=(i == 0), stop=(i == warmup_mms - 1))

            # critical prefix: x[tb0], then w0/w3 by f-group in consumption order
            load_x(0)
            for g in range(G):
                if w0_coarse:
                    nc.sync.dma_start(w0_sb[:, g], w0T[:, g])
                else:
                    for dc in range(DC):
                        nc.sync.dma_start(w0_sb[:, g, dc], w0T[:, g, dc])
                nc.sync.dma_start(w3_sb[:, g * fcg:(g + 1) * fcg],
                                  w3T[:, g * fcg:(g + 1) * fcg])

            for tb in range(NTBS):
                if tb + 1 < NTBS:
                    load_x(tb + 1)
                xt = x_tiles.pop(tb)
                # phase A: GEMM1 + GELU for all fc
                g_tiles = []
                for fc in range(FC):
                    h_ps = hpsum.tile([P, TBS], f32, name=f"h_{tb}_{fc}", tag="h_ps")
                    for dc in range(DC):
                        j = fc % fcg
                        nc.tensor.matmul(
                            h_ps[:],
                            w0_sb[:, fc // fcg, dc, j * P:(j + 1) * P],
                            xt[:, dc],
                            start=(dc == 0),
                            stop=(dc == DC - 1),
                        )
                    g_sb = gpool.tile([P, TBS], bf16, name=f"g_{tb}_{fc}", tag="g")
                    nc.scalar.activation(g_sb[:], h_ps[:], AFT.Gelu)
                    g_tiles.append(g_sb)
                # phase B: GEMM2, one [t=128, d=512] accumulation group at a time
                for ts in range(NTS):
                    for dc2 in range(2):
                        o_ps = opsum.tile([P, DW], f32, name=f"o_{tb}_{ts}_{dc2}",
                                          tag="o_ps")
                        for fc in range(FC):
                            nc.tensor.matmul(
                                o_ps[:],
                                g_tiles[fc][:, ts * P:(ts + 1) * P],
                                w3_sb[:, fc, dc2 * DW:(dc2 + 1) * DW],
                                start=(fc == 0),
                                stop=(fc == FC - 1),
                            )
                        o_sb = opool.tile([P, DW], f32, name=f"os_{tb}_{ts}_{dc2}",
                                          tag="o_sb")
                        nc.vector.tensor_copy(o_sb[:], o_ps[:])
                        nc.sync.dma_start(
                            out[tb * TBS + ts * P: tb * TBS + (ts + 1) * P,
                                dc2 * DW:(dc2 + 1) * DW],
                            o_sb[:],
                        )

    nc.compile()
    return nc


def _build_nc_v3(
    g32_bufs=4,
    h_bufs=3,
    o_ps_bufs=2,
    o_sb_bufs=3,
    x_bufs=4,
    warmup_mms=8,
):
    """fp8e4 DoubleRow variant: every operand is represented as two fp8
    digits (hi = fp8(v*s), lo = fp8(v*s - hi)) sharing one scale s per
    tensor, and each 256-wide contraction pair is computed with 3 DoubleRow
    products (hi*hi, lo*hi, hi*lo) at 0.5 cycles/row — 12 products per
    output tile vs bf16's 16 matmuls.

    Scales: x*8, w0*256 -> h_psum = h*2048, GELU applied with input scale
    1/2048; g digits unscaled (s=1); w3*256 -> out_psum = out*256, final
    copy applies 1/256.
    """
    import sys
    if "/opt/trn_rl_repo" not in sys.path:
        sys.path.insert(0, "/opt/trn_rl_repo")
    import concourse.tile as tile
    import concourse.mybir as mybir
    from concourse import bacc

    fp8 = mybir.dt.float8e4
    f32 = mybir.dt.float32
    AFT = mybir.ActivationFunctionType
    DR = mybir.MatmulPerfMode.DoubleRow
    SUB = mybir.AluOpType.subtract

    TBS = 512
    NTBS = T // TBS      # 4
    NTS = TBS // P       # 4
    DPAIR = D // 256     # 4  (256-wide contraction pairs for GEMM1)
    FPAIR = F // 256     # 16 (for GEMM2)
    G1 = 8               # w0 DMA f-groups
    FW = F // G1         # 512 f per group
    JG = FW // P         # 4 f-chunks per group

    nc = bacc.Bacc(
        "TRN2",
        target_bir_lowering=False,
        debug=False,
        enable_asserts=True,
        num_devices=N_CORES,
    )

    xh_d = nc.dram_tensor("xh", [P, NTBS, DPAIR, 2, TBS], fp8, kind="ExternalInput").ap()
    xl_d = nc.dram_tensor("xl", [P, NTBS, DPAIR, 2, TBS], fp8, kind="ExternalInput").ap()
    w0h_d = nc.dram_tensor("w0h", [P, G1, DPAIR, 2, FW], fp8, kind="ExternalInput").ap()
    w0l_d = nc.dram_tensor("w0l", [P, G1, DPAIR, 2, FW], fp8, kind="ExternalInput").ap()
    w3h_d = nc.dram_tensor("w3h", [P, FPAIR, 2, D], fp8, kind="ExternalInput").ap()
    w3l_d = nc.dram_tensor("w3l", [P, FPAIR, 2, D], fp8, kind="ExternalInput").ap()
    out = nc.dram_tensor("out", [T, D], f32, kind="ExternalOutput").ap()

    with tile.TileContext(nc) as tc:
        with (
            tc.tile_pool(name="weights", bufs=1) as wpool,
            tc.tile_pool(name="xin", bufs=x_bufs) as xpool,
            tc.tile_pool(name="g32", bufs=g32_bufs) as gpool,
            tc.tile_pool(name="ostage", bufs=o_sb_bufs) as opool,
            tc.tile_pool(name="hps", bufs=h_bufs, space="PSUM") as hpsum,
            tc.tile_pool(name="ops", bufs=o_ps_bufs, space="PSUM") as opsum,
        ):
            w0_sb = [wpool.tile([P, G1, DPAIR, 2, FW], fp8, name=f"w0{d}_sb",
                                tag=f"w0{d}") for d in "hl"]
            w3_sb = [wpool.tile([P, FPAIR, 2, D], fp8, name=f"w3{d}_sb",
                                tag=f"w3{d}") for d in "hl"]
            g_sb = [wpool.tile([P, FC, TBS], fp8, name=f"g{d}_sb",
                               tag=f"g{d}") for d in "hl"]

            x_tiles = {}
            def load_x(tb):
                xt = [xpool.tile([P, DPAIR, 2, TBS], fp8, name=f"x{d}_{tb}",
                                 tag=f"x{d}") for d in "hl"]
                nc.sync.dma_start(xt[0][:], xh_d[:, tb])
                nc.sync.dma_start(xt[1][:], xl_d[:, tb])
                x_tiles[tb] = xt

            if warmup_mms:
                with (
                    tc.tile_pool(name="warm", bufs=1) as warmpool,
                    tc.tile_pool(name="warmps", bufs=1, space="PSUM") as warmpsum,
                ):
                    wsrc = warmpool.tile([P, DW], mybir.dt.bfloat16, name="wsrc",
                                         tag="wsrc")
                    wps = warmpsum.tile([P, DW], f32, name="wps", tag="wps")
                    nc.gpsimd.memset(wsrc[:], 0.0)
                    for i in range(warmup_mms):
                        nc.tensor.matmul(wps[:], wsrc[:, :P], wsrc[:],
                                         start=(i == 0), stop=(i == warmup_mms - 1))

            # critical prefix: x(tb0) digits, then w0 f-groups in consumption
            # order, then w3 pair-groups, then remaining x blocks.
            load_x(0)
            for g in range(G1):
                nc.sync.dma_start(w0_sb[0][:, g], w0h_d[:, g])
                nc.sync.dma_start(w0_sb[1][:, g], w0l_d[:, g])
            for g in range(4):
                lo, hi = g * 4, (g + 1) * 4
                nc.sync.dma_start(w3_sb[0][:, lo:hi], w3h_d[:, lo:hi])
                nc.sync.dma_start(w3_sb[1][:, lo:hi], w3l_d[:, lo:hi])
                if g < NTBS - 1:
                    load_x(g + 1)

            for tb in range(NTBS):
                xt = x_tiles.pop(tb)
                # phase A: GEMM1 (DoubleRow, 12 products per fc) + GELU + digits
                for fc in range(FC):
                    g1, j = fc // JG, fc % JG
                    h_ps = hpsum.tile([P, TBS], f32, name=f"h_{tb}_{fc}", tag="h_ps")
                    first = True
                    for pair in range(DPAIR):
                        for (xa, wb) in ((0, 0), (1, 0), (0, 1)):
                            nc.tensor.matmul(
                                h_ps[:],
                                w0_sb[wb][:, g1, pair, :, j * P:(j + 1) * P],
                                xt[xa][:, pair],
                                start=first,
                                stop=(pair == DPAIR - 1 and (xa, wb) == (0, 1)),
                                perf_mode=DR,
                            )
                            first = False
                    g32 = gpool.tile([P, TBS], f32, name=f"g32_{tb}_{fc}", tag="g32")
                    nc.scalar.activation(g32[:], h_ps[:], AFT.Gelu,
                                         scale=1.0 / 2048.0)
                    nc.vector.tensor_copy(g_sb[0][:, fc], g32[:])
                    nc.vector.tensor_tensor(g_sb[1][:, fc], g32[:],
                                            g_sb[0][:, fc], SUB)
                # phase B: GEMM2 (DoubleRow over 16 f-pairs x 3 products)
                for ts in range(NTS):
                    for dc2 in range(2):
                        o_ps = opsum.tile([P, DW], f32, name=f"o_{tb}_{ts}_{dc2}",
                                          tag="o_ps")
                        first = True
                        for fp in range(FPAIR):
                            for (ga, wb) in ((0, 0), (1, 0), (0, 1)):
                                nc.tensor.matmul(
                                    o_ps[:],
                                    g_sb[ga][:, 2 * fp:2 * fp + 2,
                                             ts * P:(ts + 1) * P],
                                    w3_sb[wb][:, fp, :, dc2 * DW:(dc2 + 1) * DW],
                                    start=first,
                                    stop=(fp == FPAIR - 1 and (ga, wb) == (0, 1)),
                                    perf_mode=DR,
                                )
                                first = False
                        o_sb = opool.tile([P, DW], f32, name=f"os_{tb}_{ts}_{dc2}",
                                          tag="o_sb")
                        nc.scalar.activation(o_sb[:], o_ps[:], AFT.Copy,
                                             scale=1.0 / 256.0)
                        nc.sync.dma_start(
                            out[tb * TBS + ts * P: tb * TBS + (ts + 1) * P,
                                dc2 * DW:(dc2 + 1) * DW],
                            o_sb[:],
                        )

    nc.compile()
    return nc


def _get_nc():
    if "nc" not in _cache:
        _cache["nc"] = _build_nc_v3()
    return _cache["nc"]


def _make_cached_fn(nc):
    """Build a reusable jitted 8-core executable around bass2jax's bass_exec
    primitive (the same lowering run_bass_kernel_spmd uses under axon), so
    repeat kernel() calls skip retrace/relower."""
    import jax
    import numpy as np
    from jax.sharding import Mesh, PartitionSpec
    try:
        from jax.experimental.shard_map import shard_map
    except ImportError:
        from jax.shard_map import shard_map
    import concourse.mybir as mybir
    from concourse.bass2jax import (_bass_exec_p, install_neuronx_cc_hook,
                                    partition_id_tensor)

    install_neuronx_cc_hook()
    partition_name = nc.partition_id_tensor.name if nc.partition_id_tensor else None
    in_names, out_names, out_avals, zero_shapes = [], [], [], []
    for alloc in nc.m.functions[0].allocations:
        if not isinstance(alloc, mybir.MemoryLocationSet):
            continue
        name = alloc.memorylocations[0].name
        if alloc.kind == "ExternalInput":
            if name != partition_name:
                in_names.append(name)
        elif alloc.kind == "ExternalOutput":
            out_names.append(name)
            shape = tuple(alloc.tensor_shape)
            dtype = mybir.dt.np(alloc.dtype)
            out_avals.append(jax.core.ShapedArray(shape, dtype))
            zero_shapes.append((shape, dtype))
    n_params = len(in_names)
    all_in_names = list(in_names) + list(out_names)
    if partition_name is not None:
        all_in_names.append(partition_name)

    def _body(*args):
        ins = list(args[:n_params])
        outs = list(args[n_params:])
        extra = [partition_id_tensor()] if partition_name is not None else []
        return tuple(_bass_exec_p.bind(
            *ins, *outs, *extra,
            out_avals=tuple(out_avals),
            in_names=tuple(all_in_names),
            out_names=tuple(out_names),
            lowering_input_output_aliases=(),
            sim_require_finite=True,
            sim_require_nnan=True,
            nc=nc,
        ))

    devices = jax.devices()[:N_CORES]
    mesh = Mesh(np.asarray(devices), ("core",))
    fn = jax.jit(
        shard_map(_body, mesh=mesh,
                  in_specs=(PartitionSpec("core"),) * (n_params + len(out_names)),
                  out_specs=(PartitionSpec("core"),) * len(out_names),
                  check_rep=False),
        keep_unused=True)

    def run(in_maps):
        concat_in = [np.concatenate([np.asarray(m[n]) for m in in_maps], axis=0)
                     for n in in_names]
        concat_zeros = [np.zeros((N_CORES * s[0], *s[1:]), dt)
                        for s, dt in zero_shapes]
        outs = fn(*concat_in, *concat_zeros)
        return [
            {name: np.asarray(outs[i]).reshape(N_CORES, *out_avals[i].shape)[c]
             for i, name in enumerate(out_names)}
            for c in range(N_CORES)
        ]

    return run


def kernel(**inputs):
    import os
    import sys
    if "/opt/trn_rl_repo" not in sys.path:
        sys.path.insert(0, "/opt/trn_rl_repo")
    from concourse import bass_utils

    output_tensor = np.asarray(inputs["output_tensor"], dtype=np.float32)  # [1, 8]
    x = np.asarray(inputs["inputs"], dtype=np.float32)   # [1, 8, 2048, 1024]
    w0 = np.asarray(inputs["w0"], dtype=np.float32)      # [8, 4096, 1024]
    w3 = np.asarray(inputs["w3"], dtype=np.float32)      # [8, 1024, 4096]

    fp8 = ml_dtypes.float8_e4m3
    TBS, NTBS = 512, T // 512
    DPAIR, FPAIR = D // 256, F // 256
    G1 = 8
    FW = F // G1

    def split2(a):
        """hi = fp8(a), lo = fp8(a - hi): 2-digit fp8 representation."""
        hi = a.astype(fp8)
        lo = (a - hi.astype(np.float32)).astype(fp8)
        return hi, lo

    def prep_expert(e):
        # Contraction-major layouts with the 256-wide DoubleRow pair split:
        # contraction index c = pair*256 + slot*128 + ki (ki = partition).
        #   x  [128 ki, 4 tb, 4 pair, 2 slot, 512 t]   (scale 8)
        #   w0 [128 ki, 8 g, 4 pair, 2 slot, 512 fw]   (scale 256)
        #   w3 [128 ki, 16 fp, 2 slot, 1024 d]         (scale 256)
        xh, xl = split2(x[0, e].T * 8.0)          # [D, T]
        w0h, w0l = split2(w0[e].T * 256.0)        # [D, F]
        w3h, w3l = split2(w3[e].T * 256.0)        # [F, D]

        def xlay(a):
            return np.ascontiguousarray(
                a.reshape(DPAIR, 2, P, NTBS, TBS).transpose(2, 3, 0, 1, 4))

        def w0lay(a):
            return np.ascontiguousarray(
                a.reshape(DPAIR, 2, P, G1, FW).transpose(2, 3, 0, 1, 4))

        def w3lay(a):
            return np.ascontiguousarray(
                a.reshape(FPAIR, 2, P, D).transpose(2, 0, 1, 3))

        return {
            "xh": xlay(xh), "xl": xlay(xl),
            "w0h": w0lay(w0h), "w0l": w0lay(w0l),
            "w3h": w3lay(w3h), "w3l": w3lay(w3l),
        }

    from concurrent.futures import ThreadPoolExecutor
    with ThreadPoolExecutor(max_workers=N_CORES) as pool:
        in_maps = list(pool.map(prep_expert, range(N_CORES)))

    nc = _get_nc()
    results = None
    if "fast_fn" in _cache:
        try:
            results = _cache["fast_fn"](in_maps)
        except Exception:
            results = None
    if results is None:
        try:
            results = bass_utils.run_bass_kernel_spmd(
                nc, in_maps, core_ids=list(range(N_CORES))).results
        except ModuleNotFoundError:
            # trace path requested via env but axon NTFF hook missing
            os.environ["BASS_NEVER_TRACE"] = "1"
            results = bass_utils.run_bass_kernel_spmd(
                nc, in_maps, core_ids=list(range(N_CORES))).results
        try:
            fast = _make_cached_fn(nc)
            fast(in_maps)  # warm: jit trace + XLA/NEFF compile happens here
            _cache["fast_fn"] = fast
        except Exception:
            pass
    out_full = np.stack([results[e]["out"] for e in range(N_CORES)])[None]

    # unpopular experts with zero gating activity produce zeros
    unpop = output_tensor[:, NUM_LOCAL:].sum(axis=0) != 0
    mask = np.concatenate([np.ones(NUM_LOCAL, dtype=bool), unpop])
    out_full = out_full * mask[None, :, None, None].astype(np.float32)
    return out_full.astype(np.float32)



# revision 11
# speedup vs baseline: 1.3698x; 1.0449x over previous
"""Trainium2 Bass kernel for nn_Experts (grouped MoE expert MLP).

Computes, for each of 8 experts e:
    h   = x_e @ w0_e.T          # [2048,1024] @ [1024,4096] -> [2048,4096]
    g   = gelu_exact(h)
    out = g @ w3_e.T            # [2048,4096] @ [4096,1024] -> [2048,1024]
then masks unpopular experts with zero gating activity (output_tensor).

Sharding: expert-parallel, 1 expert per NeuronCore across 8 cores (SPMD —
one compiled NEFF, per-core input data).

Layout strategy: all operands are pre-transposed on the host into
contraction-major ("K-major") layouts so the device kernel needs no
transposes at all:
    xT  [128, 8, 2048]  (d%128, d//128, t)   bf16
    w0T [128, 8, 4096]  (d%128, d//128, f)   bf16
    w3T [128, 32, 1024] (f%128, f//128, d)   bf16
GEMM1 produces hT tiles [f=128, t] in PSUM, GELU moves them to SBUF as bf16,
and those tiles are directly the lhsT operand of GEMM2 (contraction over f),
whose PSUM output [t=128, d] accumulates over all 32 f-chunks and lands in
the natural [t, d] layout of the output.
"""

import numpy as np
import ml_dtypes

T = 2048      # tokens (capacity) per expert
D = 1024      # hidden
F = 4096      # ffn
P = 128       # partitions
TB = 256      # token block (GEMM1 moving free dim)
NTB = T // TB
DC = D // P   # 8 d-chunks (GEMM1 contraction)
FC = F // P   # 32 f-chunks (GEMM2 contraction)
DW = 512      # GEMM2 output free-dim chunk
NUM_LOCAL = 4
N_CORES = 8

_cache = {}


def _build_nc(
    tb_size=TB,          # token block
    x_split=1,           # extra splits of each x d-chunk DMA (along t)
    w0_split=1,          # extra splits of each w0 d-chunk DMA (along f)
    w3_group=1,          # f-chunks per w3 DMA
    g_bufs=4,
    h_bufs=2,
    o_sb_bufs=4,
    dma_scheme="tuned",  # "simple" | "tuned" (critical-prefix-first ordering)
    fcg=4,               # fc per w0/w3 DMA group in tuned scheme
    pipeline_o=True,     # issue GEMM2(fc) after GEMM1(fc+1) to hide gelu latency
):
    import sys
    if "/opt/trn_rl_repo" not in sys.path:
        sys.path.insert(0, "/opt/trn_rl_repo")
    import concourse.bass as bass
    import concourse.tile as tile
    import concourse.mybir as mybir
    from concourse import bacc

    bf16 = mybir.dt.bfloat16
    f32 = mybir.dt.float32
    AFT = mybir.ActivationFunctionType

    TBS = tb_size
    NTBS = T // TBS
    NTS = TBS // P       # t-subchunks per block (GEMM2 lhsT count)
    n_ops = NTS * 2      # out psum tiles per block ([t 128] x [d 512])

    nc = bacc.Bacc(
        "TRN2",
        target_bir_lowering=False,
        debug=False,
        enable_asserts=True,
        num_devices=N_CORES,
    )

    xT = nc.dram_tensor("xT", [P, DC, T], bf16, kind="ExternalInput").ap()
    w0T = nc.dram_tensor("w0T", [P, DC, F], bf16, kind="ExternalInput").ap()
    w3T = nc.dram_tensor("w3T", [P, FC, D], bf16, kind="ExternalInput").ap()
    out = nc.dram_tensor("out", [T, D], f32, kind="ExternalOutput").ap()

    with tile.TileContext(nc) as tc:
        with (
            tc.tile_pool(name="weights", bufs=1) as wpool,
            tc.tile_pool(name="gelu", bufs=g_bufs) as gpool,
            tc.tile_pool(name="ostage", bufs=o_sb_bufs) as opool,
            tc.tile_pool(name="hps", bufs=h_bufs, space="PSUM") as hpsum,
            tc.tile_pool(name="ops", bufs=n_ops, space="PSUM") as opsum,
        ):
            x_sb = wpool.tile([P, DC, T], bf16, name="x_sb", tag="x_sb")
            w0_sb = wpool.tile([P, DC, F], bf16, name="w0_sb", tag="w0_sb")
            w3_sb = wpool.tile([P, FC, D], bf16, name="w3_sb", tag="w3_sb")

            if dma_scheme == "simple":
                # Load x and w0 first (first h-tile needs ALL d-chunks of
                # both); w3 f-chunks stream in behind.
                for dc in range(DC):
                    for s in range(x_split):
                        w = T // x_split
                        nc.sync.dma_start(x_sb[:, dc, s * w:(s + 1) * w],
                                          xT[:, dc, s * w:(s + 1) * w])
                    for s in range(w0_split):
                        w = F // w0_split
                        nc.sync.dma_start(w0_sb[:, dc, s * w:(s + 1) * w],
                                          w0T[:, dc, s * w:(s + 1) * w])
                for g in range(FC // w3_group):
                    lo, hi = g * w3_group, (g + 1) * w3_group
                    nc.sync.dma_start(w3_sb[:, lo:hi], w3T[:, lo:hi])
            else:
                # Critical-prefix-first: x for tb0, then per-f-group w0 (all
                # d-chunks) and w3 interleaved in the order GEMM1/GEMM2
                # consume them, then the rest of x.
                for dc in range(DC):
                    nc.sync.dma_start(x_sb[:, dc, 0:TBS], xT[:, dc, 0:TBS])
                for g in range(FC // fcg):
                    flo, fhi = g * fcg * P, (g + 1) * fcg * P
                    for dc in range(DC):
                        nc.sync.dma_start(w0_sb[:, dc, flo:fhi],
                                          w0T[:, dc, flo:fhi])
                    nc.sync.dma_start(w3_sb[:, g * fcg:(g + 1) * fcg],
                                      w3T[:, g * fcg:(g + 1) * fcg])
                for tb in range(1, NTBS):
                    for dc in range(DC):
                        nc.sync.dma_start(
                            x_sb[:, dc, tb * TBS:(tb + 1) * TBS],
                            xT[:, dc, tb * TBS:(tb + 1) * TBS])

            for tb in range(NTBS):
                o_ps = [
                    opsum.tile([P, DW], f32, name=f"o_ps_{tb}_{i}", tag="o_ps")
                    for i in range(n_ops)
                ]

                def emit_o(fc, g_sb):
                    for ts in range(NTS):
                        for dc2 in range(2):
                            nc.tensor.matmul(
                                o_ps[ts * 2 + dc2][:],
                                g_sb[:, ts * P:(ts + 1) * P],
                                w3_sb[:, fc, dc2 * DW:(dc2 + 1) * DW],
                                start=(fc == 0),
                                stop=(fc == FC - 1),
                            )

                pending = None
                for fc in range(FC):
                    h_ps = hpsum.tile([P, TBS], f32, name=f"h_ps_{tb}_{fc}", tag="h_ps")
                    for dc in range(DC):
                        nc.tensor.matmul(
                            h_ps[:],
                            w0_sb[:, dc, fc * P:(fc + 1) * P],
                            x_sb[:, dc, tb * TBS:(tb + 1) * TBS],
                            start=(dc == 0),
                            stop=(dc == DC - 1),
                        )
                    g_sb = gpool.tile([P, TBS], bf16, name=f"g_{tb}_{fc}", tag="g")
                    nc.scalar.activation(g_sb[:], h_ps[:], AFT.Gelu)
                    if not pipeline_o:
                        emit_o(fc, g_sb)
                    else:
                        if pending is not None:
                            emit_o(*pending)
                        pending = (fc, g_sb)
                if pending is not None:
                    emit_o(*pending)

                for ts in range(NTS):
                    for dc2 in range(2):
                        o_sb = opool.tile([P, DW], f32, name=f"o_sb_{tb}_{ts}_{dc2}",
                                          tag="o_sb")
                        nc.vector.tensor_copy(o_sb[:], o_ps[ts * 2 + dc2][:])
                        nc.sync.dma_start(
                            out[tb * TBS + ts * P: tb * TBS + (ts + 1) * P,
                                dc2 * DW:(dc2 + 1) * DW],
                            o_sb[:],
                        )

    nc.compile()
    return nc


def _build_nc_v2(
    g_extra=0,           # extra gelu-tile slots beyond FC (lookahead into next block)
    h_bufs=3,
    o_ps_bufs=2,
    o_sb_bufs=3,
    x_bufs=2,
    fcg=4,               # fc per w0/w3 DMA group
    x_coarse=True,       # one DMA per x block vs per-dc
    w0_coarse=False,     # one DMA per w0 f-group vs per-dc
    warmup_mms=8,        # scratch matmuls issued before the real work so the
                         # PE rides out the HAM cold-clock window during the
                         # initial DMA wait instead of during real matmuls
):
    """TB=512 two-phase variant: per 512-token block, phase A runs GEMM1+GELU
    for all 32 f-chunks (g tiles [128,512] bf16 stay in SBUF), phase B runs
    GEMM2 as 8 sequential PSUM accumulation groups (one [t=128, d=512] output
    tile each, contraction over all 32 f-chunks). x is streamed per-block
    instead of fully resident to stay under the SBUF cap."""
    import sys
    if "/opt/trn_rl_repo" not in sys.path:
        sys.path.insert(0, "/opt/trn_rl_repo")
    import concourse.tile as tile
    import concourse.mybir as mybir
    from concourse import bacc

    bf16 = mybir.dt.bfloat16
    f32 = mybir.dt.float32
    AFT = mybir.ActivationFunctionType

    TBS = 512
    NTBS = T // TBS      # 4
    NTS = TBS // P       # 4

    G = FC // fcg        # w0 DMA groups
    FW = fcg * P         # f elements per group (512)

    nc = bacc.Bacc(
        "TRN2",
        target_bir_lowering=False,
        debug=False,
        enable_asserts=True,
        num_devices=N_CORES,
    )

    # DRAM layouts are grouped so every load has long (8KB) contiguous
    # per-partition runs: xT by token-block, w0T by f-group.
    xT = nc.dram_tensor("xT", [P, NTBS, DC, TBS], bf16, kind="ExternalInput").ap()
    w0T = nc.dram_tensor("w0T", [P, G, DC, FW], bf16, kind="ExternalInput").ap()
    w3T = nc.dram_tensor("w3T", [P, FC, D], bf16, kind="ExternalInput").ap()
    out = nc.dram_tensor("out", [T, D], f32, kind="ExternalOutput").ap()

    with tile.TileContext(nc) as tc:
        with (
            tc.tile_pool(name="weights", bufs=1) as wpool,
            tc.tile_pool(name="xin", bufs=x_bufs) as xpool,
            tc.tile_pool(name="gelu", bufs=FC + g_extra) as gpool,
            tc.tile_pool(name="ostage", bufs=o_sb_bufs) as opool,
            tc.tile_pool(name="hps", bufs=h_bufs, space="PSUM") as hpsum,
            tc.tile_pool(name="ops", bufs=o_ps_bufs, space="PSUM") as opsum,
        ):
            # w0 SBUF mirrors the grouped DRAM layout; GEMM1 slices
            # [:, fc//fcg, dc, (fc%fcg)*P : +P].
            w0_sb = wpool.tile([P, G, DC, FW], bf16, name="w0_sb", tag="w0_sb")
            w3_sb = wpool.tile([P, FC, D], bf16, name="w3_sb", tag="w3_sb")

            x_tiles = {}
            def load_x(tb):
                xt = xpool.tile([P, DC, TBS], bf16, name=f"x_{tb}", tag="x")
                if x_coarse:
                    nc.sync.dma_start(xt[:], xT[:, tb])
                else:
                    for dc in range(DC):
                        nc.sync.dma_start(xt[:, dc], xT[:, tb, dc])
                x_tiles[tb] = xt

            if warmup_mms:
                with (
                    tc.tile_pool(name="warm", bufs=1) as warmpool,
                    tc.tile_pool(name="warmps", bufs=1, space="PSUM") as warmpsum,
                ):
                    wsrc = warmpool.tile([P, DW], bf16, name="wsrc", tag="wsrc")
                    wps = warmpsum.tile([P, DW], f32, name="wps", tag="wps")
                    nc.gpsimd.memset(wsrc[:], 0.0)
                    for i in range(warmup_mms):
                        nc.tensor.matmul(wps[:], wsrc[:, :P], wsrc[:],
                                         start=(i == 0), stop=(i == warmup_mms - 1))

            # critical prefix: x[tb0], then w0/w3 by f-group in consumption order
            load_x(0)
            for g in range(G):
                if w0_coarse:
                    nc.sync.dma_start(w0_sb[:, g], w0T[:, g])
                else:
                    for dc in range(DC):
                        nc.sync.dma_start(w0_sb[:, g, dc], w0T[:, g, dc])
                nc.sync.dma_start(w3_sb[:, g * fcg:(g + 1) * fcg],
                                  w3T[:, g * fcg:(g + 1) * fcg])

            for tb in range(NTBS):
                if tb + 1 < NTBS:
                    load_x(tb + 1)
                xt = x_tiles.pop(tb)
                # phase A: GEMM1 + GELU for all fc
                g_tiles = []
                for fc in range(FC):
                    h_ps = hpsum.tile([P, TBS], f32, name=f"h_{tb}_{fc}", tag="h_ps")
                    for dc in range(DC):
                        j = fc % fcg
                        nc.tensor.matmul(
                            h_ps[:],
                            w0_sb[:, fc // fcg, dc, j * P:(j + 1) * P],
                            xt[:, dc],
                            start=(dc == 0),
                            stop=(dc == DC - 1),
                        )
                    g_sb = gpool.tile([P, TBS], bf16, name=f"g_{tb}_{fc}", tag="g")
                    nc.scalar.activation(g_sb[:], h_ps[:], AFT.Gelu)
                    g_tiles.append(g_sb)
                # phase B: GEMM2, one [t=128, d=512] accumulation group at a time
                for ts in range(NTS):
                    for dc2 in range(2):
                        o_ps = opsum.tile([P, DW], f32, name=f"o_{tb}_{ts}_{dc2}",
                                          tag="o_ps")
                        for fc in range(FC):
                            nc.tensor.matmul(
                                o_ps[:],
                                g_tiles[fc][:, ts * P:(ts + 1) * P],
                                w3_sb[:, fc, dc2 * DW:(dc2 + 1) * DW],
                                start=(fc == 0),
                                stop=(fc == FC - 1),
                            )
                        o_sb = opool.tile([P, DW], f32, name=f"os_{tb}_{ts}_{dc2}",
                                          tag="o_sb")
                        nc.vector.tensor_copy(o_sb[:], o_ps[:])
                        nc.sync.dma_start(
                            out[tb * TBS + ts * P: tb * TBS + (ts + 1) * P,
                                dc2 * DW:(dc2 + 1) * DW],
                            o_sb[:],
                        )

    nc.compile()
    return nc


def _build_nc_v3(
    g32_bufs=4,
    h_bufs=3,
    o_ps_bufs=2,
    o_sb_bufs=3,
    x_bufs=4,
    warmup_mms=8,
    x_res_pairs=3,     # d-pairs (of 4) that get the x_lo@w0_hi product
    g_res_fpairs=16,   # f-pairs (of 16) that get the g_lo@w3_hi product
    w3_res_fpairs=16,  # f-pairs (of 16) that get the g_hi@w3_lo product
):
    """fp8e4 DoubleRow variant: every operand is represented as two fp8
    digits (hi = fp8(v*s), lo = fp8(v*s - hi)) sharing one scale s per
    tensor, and each 256-wide contraction pair is computed with 3 DoubleRow
    products (hi*hi, lo*hi, hi*lo) at 0.5 cycles/row — 12 products per
    output tile vs bf16's 16 matmuls.

    Scales: x*8, w0*256 -> h_psum = h*2048, GELU applied with input scale
    1/2048; g digits unscaled (s=1); w3*256 -> out_psum = out*256, final
    copy applies 1/256.
    """
    import sys
    if "/opt/trn_rl_repo" not in sys.path:
        sys.path.insert(0, "/opt/trn_rl_repo")
    import concourse.tile as tile
    import concourse.mybir as mybir
    from concourse import bacc

    fp8 = mybir.dt.float8e4
    f32 = mybir.dt.float32
    AFT = mybir.ActivationFunctionType
    DR = mybir.MatmulPerfMode.DoubleRow
    SUB = mybir.AluOpType.subtract

    TBS = 512
    NTBS = T // TBS      # 4
    NTS = TBS // P       # 4
    DPAIR = D // 256     # 4  (256-wide contraction pairs for GEMM1)
    FPAIR = F // 256     # 16 (for GEMM2)
    G1 = 16              # w0 DMA f-groups (small groups -> short critical prefix)
    FW = F // G1         # 256 f per group
    JG = FW // P         # 2 f-chunks per group

    nc = bacc.Bacc(
        "TRN2",
        target_bir_lowering=False,
        debug=False,
        enable_asserts=True,
        num_devices=N_CORES,
    )

    xh_d = nc.dram_tensor("xh", [P, NTBS, DPAIR, 2, TBS], fp8, kind="ExternalInput").ap()
    xl_d = nc.dram_tensor("xl", [P, NTBS, DPAIR, 2, TBS], fp8, kind="ExternalInput").ap()
    w0h_d = nc.dram_tensor("w0h", [P, G1, DPAIR, 2, FW], fp8, kind="ExternalInput").ap()
    w0l_d = nc.dram_tensor("w0l", [P, G1, DPAIR, 2, FW], fp8, kind="ExternalInput").ap()
    w3h_d = nc.dram_tensor("w3h", [P, FPAIR, 2, D], fp8, kind="ExternalInput").ap()
    w3l_d = nc.dram_tensor("w3l", [P, FPAIR, 2, D], fp8, kind="ExternalInput").ap()
    out = nc.dram_tensor("out", [T, D], f32, kind="ExternalOutput").ap()

    with tile.TileContext(nc) as tc:
        with (
            tc.tile_pool(name="weights", bufs=1) as wpool,
            tc.tile_pool(name="xin", bufs=x_bufs) as xpool,
            tc.tile_pool(name="g32", bufs=g32_bufs) as gpool,
            tc.tile_pool(name="ostage", bufs=o_sb_bufs) as opool,
            tc.tile_pool(name="hps", bufs=h_bufs, space="PSUM") as hpsum,
            tc.tile_pool(name="ops", bufs=o_ps_bufs, space="PSUM") as opsum,
        ):
            w0_sb = [wpool.tile([P, G1, DPAIR, 2, FW], fp8, name=f"w0{d}_sb",
                                tag=f"w0{d}") for d in "hl"]
            w3_sb = [wpool.tile([P, FPAIR, 2, D], fp8, name=f"w3{d}_sb",
                                tag=f"w3{d}") for d in "hl"]
            g_sb = [wpool.tile([P, FC, TBS], fp8, name=f"g{d}_sb",
                               tag=f"g{d}") for d in "hl"]

            x_tiles = {}
            def load_x(tb, digits=(0, 1)):
                xt = x_tiles.setdefault(tb, [None, None])
                for dgt in digits:
                    t = xpool.tile([P, DPAIR, 2, TBS], fp8,
                                   name=f"x{'hl'[dgt]}_{tb}", tag=f"x{'hl'[dgt]}")
                    nc.sync.dma_start(t[:], (xh_d, xl_d)[dgt][:, tb])
                    xt[dgt] = t

            if warmup_mms:
                with (
                    tc.tile_pool(name="warm", bufs=1) as warmpool,
                    tc.tile_pool(name="warmps", bufs=1, space="PSUM") as warmpsum,
                ):
                    wsrc = warmpool.tile([P, DW], mybir.dt.bfloat16, name="wsrc",
                                         tag="wsrc")
                    wps = warmpsum.tile([P, DW], f32, name="wps", tag="wps")
                    nc.gpsimd.memset(wsrc[:], 0.0)
                    for i in range(warmup_mms):
                        nc.tensor.matmul(wps[:], wsrc[:, :P], wsrc[:],
                                         start=(i == 0), stop=(i == warmup_mms - 1))

            # critical prefix in first-consumption order (products within an
            # fc run digit-major hh, lh, hl): xh(tb0), w0h g0, xl, w0l g0,
            # then remaining w0 groups, then w3 groups + remaining x blocks.
            load_x(0, digits=(0,))
            nc.sync.dma_start(w0_sb[0][:, 0], w0h_d[:, 0])
            load_x(0, digits=(1,))
            nc.sync.dma_start(w0_sb[1][:, 0], w0l_d[:, 0])
            for g in range(1, G1):
                nc.sync.dma_start(w0_sb[0][:, g], w0h_d[:, g])
                nc.sync.dma_start(w0_sb[1][:, g], w0l_d[:, g])
            for g in range(4):
                lo, hi = g * 4, (g + 1) * 4
                nc.sync.dma_start(w3_sb[0][:, lo:hi], w3h_d[:, lo:hi])
                nc.sync.dma_start(w3_sb[1][:, lo:hi], w3l_d[:, lo:hi])
                if g < NTBS - 1:
                    load_x(g + 1)

            for tb in range(NTBS):
                xt = x_tiles.pop(tb)
                # phase A: GEMM1 (DoubleRow, 12 products per fc) + GELU + digits
                for fc in range(FC):
                    g1, j = fc // JG, fc % JG
                    h_ps = hpsum.tile([P, TBS], f32, name=f"h_{tb}_{fc}", tag="h_ps")
                    prods = ([(p, 0, 0) for p in range(DPAIR)]
                             + [(p, 1, 0) for p in range(x_res_pairs)]
                             + [(p, 0, 1) for p in range(DPAIR)])
                    for i, (pair, xa, wb) in enumerate(prods):
                        nc.tensor.matmul(
                            h_ps[:],
                            w0_sb[wb][:, g1, pair, :, j * P:(j + 1) * P],
                            xt[xa][:, pair],
                            start=(i == 0),
                            stop=(i == len(prods) - 1),
                            perf_mode=DR,
                        )
                    g32 = gpool.tile([P, TBS], f32, name=f"g32_{tb}_{fc}", tag="g32")
                    nc.scalar.activation(g32[:], h_ps[:], AFT.Gelu,
                                         scale=1.0 / 2048.0)
                    nc.vector.tensor_copy(g_sb[0][:, fc], g32[:])
                    nc.vector.tensor_tensor(g_sb[1][:, fc], g32[:],
                                            g_sb[0][:, fc], SUB)
                # phase B: GEMM2 (DoubleRow over 16 f-pairs x 3 products)
                for ts in range(NTS):
                    for dc2 in range(2):
                        o_ps = opsum.tile([P, DW], f32, name=f"o_{tb}_{ts}_{dc2}",
                                          tag="o_ps")
                        prods = []
                        for fp in range(FPAIR):
                            prods.append((fp, 0, 0))
                            if fp < g_res_fpairs:
                                prods.append((fp, 1, 0))
                            if fp < w3_res_fpairs:
                                prods.append((fp, 0, 1))
                        for i, (fp, ga, wb) in enumerate(prods):
                            nc.tensor.matmul(
                                o_ps[:],
                                g_sb[ga][:, 2 * fp:2 * fp + 2,
                                         ts * P:(ts + 1) * P],
                                w3_sb[wb][:, fp, :, dc2 * DW:(dc2 + 1) * DW],
                                start=(i == 0),
                                stop=(i == len(prods) - 1),
                                perf_mode=DR,
                            )
                        o_sb = opool.tile([P, DW], f32, name=f"os_{tb}_{ts}_{dc2}",
                                          tag="o_sb")
                        nc.scalar.activation(o_sb[:], o_ps[:], AFT.Copy,
                                             scale=1.0 / 256.0)
                        nc.sync.dma_start(
                            out[tb * TBS + ts * P: tb * TBS + (ts + 1) * P,
                                dc2 * DW:(dc2 + 1) * DW],
                            o_sb[:],
                        )

    nc.compile()
    return nc


def _get_nc():
    if "nc" not in _cache:
        _cache["nc"] = _build_nc_v3()
    return _cache["nc"]


def _make_cached_fn(nc):
    """Build a reusable jitted 8-core executable around bass2jax's bass_exec
    primitive (the same lowering run_bass_kernel_spmd uses under axon), so
    repeat kernel() calls skip retrace/relower."""
    import jax
    import numpy as np
    from jax.sharding import Mesh, PartitionSpec
    try:
        from jax.experimental.shard_map import shard_map
    except ImportError:
        from jax.shard_map import shard_map
    import concourse.mybir as mybir
    from concourse.bass2jax import (_bass_exec_p, install_neuronx_cc_hook,
                                    partition_id_tensor)

    install_neuronx_cc_hook()
    partition_name = nc.partition_id_tensor.name if nc.partition_id_tensor else None
    in_names, out_names, out_avals, zero_shapes = [], [], [], []
    for alloc in nc.m.functions[0].allocations:
        if not isinstance(alloc, mybir.MemoryLocationSet):
            continue
        name = alloc.memorylocations[0].name
        if alloc.kind == "ExternalInput":
            if name != partition_name:
                in_names.append(name)
        elif alloc.kind == "ExternalOutput":
            out_names.append(name)
            shape = tuple(alloc.tensor_shape)
            dtype = mybir.dt.np(alloc.dtype)
            out_avals.append(jax.core.ShapedArray(shape, dtype))
            zero_shapes.append((shape, dtype))
    n_params = len(in_names)
    all_in_names = list(in_names) + list(out_names)
    if partition_name is not None:
        all_in_names.append(partition_name)

    def _body(*args):
        ins = list(args[:n_params])
        outs = list(args[n_params:])
        extra = [partition_id_tensor()] if partition_name is not None else []
        return tuple(_bass_exec_p.bind(
            *ins, *outs, *extra,
            out_avals=tuple(out_avals),
            in_names=tuple(all_in_names),
            out_names=tuple(out_names),
            lowering_input_output_aliases=(),
            sim_require_finite=True,
            sim_require_nnan=True,
            nc=nc,
        ))

    devices = jax.devices()[:N_CORES]
    mesh = Mesh(np.asarray(devices), ("core",))
    fn = jax.jit(
        shard_map(_body, mesh=mesh,
                  in_specs=(PartitionSpec("core"),) * (n_params + len(out_names)),
                  out_specs=(PartitionSpec("core"),) * len(out_names),
                  check_rep=False),
        keep_unused=True)

    def run(in_maps):
        concat_in = [np.concatenate([np.asarray(m[n]) for m in in_maps], axis=0)
                     for n in in_names]
        concat_zeros = [np.zeros((N_CORES * s[0], *s[1:]), dt)
                        for s, dt in zero_shapes]
        outs = fn(*concat_in, *concat_zeros)
        return [
            {name: np.asarray(outs[i]).reshape(N_CORES, *out_avals[i].shape)[c]
             for i, name in enumerate(out_names)}
            for c in range(N_CORES)
        ]

    return run


def kernel(**inputs):
    import os
    import sys
    if "/opt/trn_rl_repo" not in sys.path:
        sys.path.insert(0, "/opt/trn_rl_repo")
    from concourse import bass_utils

    output_tensor = np.asarray(inputs["output_tensor"], dtype=np.float32)  # [1, 8]
    x = np.asarray(inputs["inputs"], dtype=np.float32)   # [1, 8, 2048, 1024]
    w0 = np.asarray(inputs["w0"], dtype=np.float32)      # [8, 4096, 1024]
    w3 = np.asarray(inputs["w3"], dtype=np.float32)      # [8, 1024, 4096]

    fp8 = ml_dtypes.float8_e4m3
    TBS, NTBS = 512, T // 512
    DPAIR, FPAIR = D // 256, F // 256
    G1 = 16
    FW = F // G1

    def split2(a):
        """hi = fp8(a), lo = fp8(a - hi): 2-digit fp8 representation."""
        hi = a.astype(fp8)
        lo = (a - hi.astype(np.float32)).astype(fp8)
        return hi, lo

    def prep_expert(e):
        # Contraction-major layouts with the 256-wide DoubleRow pair split:
        # contraction index c = pair*256 + slot*128 + ki (ki = partition).
        #   x  [128 ki, 4 tb, 4 pair, 2 slot, 512 t]   (scale 8)
        #   w0 [128 ki, 8 g, 4 pair, 2 slot, 512 fw]   (scale 256)
        #   w3 [128 ki, 16 fp, 2 slot, 1024 d]         (scale 256)
        xh, xl = split2(x[0, e].T * 8.0)          # [D, T]
        w0h, w0l = split2(w0[e].T * 256.0)        # [D, F]
        w3h, w3l = split2(w3[e].T * 256.0)        # [F, D]

        def xlay(a):
            return np.ascontiguousarray(
                a.reshape(DPAIR, 2, P, NTBS, TBS).transpose(2, 3, 0, 1, 4))

        def w0lay(a):
            return np.ascontiguousarray(
                a.reshape(DPAIR, 2, P, G1, FW).transpose(2, 3, 0, 1, 4))

        def w3lay(a):
            return np.ascontiguousarray(
                a.reshape(FPAIR, 2, P, D).transpose(2, 0, 1, 3))

        return {
            "xh": xlay(xh), "xl": xlay(xl),
            "w0h": w0lay(w0h), "w0l": w0lay(w0l),
            "w3h": w3lay(w3h), "w3l": w3lay(w3l),
        }

    from concurrent.futures import ThreadPoolExecutor
    with ThreadPoolExecutor(max_workers=N_CORES) as pool:
        in_maps = list(pool.map(prep_expert, range(N_CORES)))

    nc = _get_nc()
    results = None
    if "fast_fn" in _cache:
        try:
            results = _cache["fast_fn"](in_maps)
        except Exception:
            results = None
    if results is None:
        try:
            results = bass_utils.run_bass_kernel_spmd(
                nc, in_maps, core_ids=list(range(N_CORES))).results
        except ModuleNotFoundError:
            # trace path requested via env but axon NTFF hook missing
            os.environ["BASS_NEVER_TRACE"] = "1"
            results = bass_utils.run_bass_kernel_spmd(
                nc, in_maps, core_ids=list(range(N_CORES))).results
        try:
            fast = _make_cached_fn(nc)
            fast(in_maps)  # warm: jit trace + XLA/NEFF compile happens here
            _cache["fast_fn"] = fast
        except Exception:
            pass
    out_full = np.stack([results[e]["out"] for e in range(N_CORES)])[None]

    # unpopular experts with zero gating activity produce zeros
    unpop = output_tensor[:, NUM_LOCAL:].sum(axis=0) != 0
    mask = np.concatenate([np.ones(NUM_LOCAL, dtype=bool), unpop])
    out_full = out_full * mask[None, :, None, None].astype(np.float32)
    return out_full.astype(np.float32)



# revision 14
# speedup vs baseline: 1.3965x; 1.0196x over previous
"""Trainium2 Bass kernel for nn_Experts (grouped MoE expert MLP).

Computes, for each of 8 experts e:
    h   = x_e @ w0_e.T          # [2048,1024] @ [1024,4096] -> [2048,4096]
    g   = gelu_exact(h)
    out = g @ w3_e.T            # [2048,4096] @ [4096,1024] -> [2048,1024]
then masks unpopular experts with zero gating activity (output_tensor).

Sharding: expert-parallel, 1 expert per NeuronCore across 8 cores (SPMD —
one compiled NEFF, per-core input data).

Layout strategy: all operands are pre-transposed on the host into
contraction-major ("K-major") layouts so the device kernel needs no
transposes at all:
    xT  [128, 8, 2048]  (d%128, d//128, t)   bf16
    w0T [128, 8, 4096]  (d%128, d//128, f)   bf16
    w3T [128, 32, 1024] (f%128, f//128, d)   bf16
GEMM1 produces hT tiles [f=128, t] in PSUM, GELU moves them to SBUF as bf16,
and those tiles are directly the lhsT operand of GEMM2 (contraction over f),
whose PSUM output [t=128, d] accumulates over all 32 f-chunks and lands in
the natural [t, d] layout of the output.
"""

import numpy as np
import ml_dtypes

T = 2048      # tokens (capacity) per expert
D = 1024      # hidden
F = 4096      # ffn
P = 128       # partitions
TB = 256      # token block (GEMM1 moving free dim)
NTB = T // TB
DC = D // P   # 8 d-chunks (GEMM1 contraction)
FC = F // P   # 32 f-chunks (GEMM2 contraction)
DW = 512      # GEMM2 output free-dim chunk
NUM_LOCAL = 4
N_CORES = 8

_cache = {}


def _build_nc(
    tb_size=TB,          # token block
    x_split=1,           # extra splits of each x d-chunk DMA (along t)
    w0_split=1,          # extra splits of each w0 d-chunk DMA (along f)
    w3_group=1,          # f-chunks per w3 DMA
    g_bufs=4,
    h_bufs=2,
    o_sb_bufs=4,
    dma_scheme="tuned",  # "simple" | "tuned" (critical-prefix-first ordering)
    fcg=4,               # fc per w0/w3 DMA group in tuned scheme
    pipeline_o=True,     # issue GEMM2(fc) after GEMM1(fc+1) to hide gelu latency
):
    import sys
    if "/opt/trn_rl_repo" not in sys.path:
        sys.path.insert(0, "/opt/trn_rl_repo")
    import concourse.bass as bass
    import concourse.tile as tile
    import concourse.mybir as mybir
    from concourse import bacc

    bf16 = mybir.dt.bfloat16
    f32 = mybir.dt.float32
    AFT = mybir.ActivationFunctionType

    TBS = tb_size
    NTBS = T // TBS
    NTS = TBS // P       # t-subchunks per block (GEMM2 lhsT count)
    n_ops = NTS * 2      # out psum tiles per block ([t 128] x [d 512])

    nc = bacc.Bacc(
        "TRN2",
        target_bir_lowering=False,
        debug=False,
        enable_asserts=True,
        num_devices=N_CORES,
    )

    xT = nc.dram_tensor("xT", [P, DC, T], bf16, kind="ExternalInput").ap()
    w0T = nc.dram_tensor("w0T", [P, DC, F], bf16, kind="ExternalInput").ap()
    w3T = nc.dram_tensor("w3T", [P, FC, D], bf16, kind="ExternalInput").ap()
    out = nc.dram_tensor("out", [T, D], f32, kind="ExternalOutput").ap()

    with tile.TileContext(nc) as tc:
        with (
            tc.tile_pool(name="weights", bufs=1) as wpool,
            tc.tile_pool(name="gelu", bufs=g_bufs) as gpool,
            tc.tile_pool(name="ostage", bufs=o_sb_bufs) as opool,
            tc.tile_pool(name="hps", bufs=h_bufs, space="PSUM") as hpsum,
            tc.tile_pool(name="ops", bufs=n_ops, space="PSUM") as opsum,
        ):
            x_sb = wpool.tile([P, DC, T], bf16, name="x_sb", tag="x_sb")
            w0_sb = wpool.tile([P, DC, F], bf16, name="w0_sb", tag="w0_sb")
            w3_sb = wpool.tile([P, FC, D], bf16, name="w3_sb", tag="w3_sb")

            if dma_scheme == "simple":
                # Load x and w0 first (first h-tile needs ALL d-chunks of
                # both); w3 f-chunks stream in behind.
                for dc in range(DC):
                    for s in range(x_split):
                        w = T // x_split
                        nc.sync.dma_start(x_sb[:, dc, s * w:(s + 1) * w],
                                          xT[:, dc, s * w:(s + 1) * w])
                    for s in range(w0_split):
                        w = F // w0_split
                        nc.sync.dma_start(w0_sb[:, dc, s * w:(s + 1) * w],
                                          w0T[:, dc, s * w:(s + 1) * w])
                for g in range(FC // w3_group):
                    lo, hi = g * w3_group, (g + 1) * w3_group
                    nc.sync.dma_start(w3_sb[:, lo:hi], w3T[:, lo:hi])
            else:
                # Critical-prefix-first: x for tb0, then per-f-group w0 (all
                # d-chunks) and w3 interleaved in the order GEMM1/GEMM2
                # consume them, then the rest of x.
                for dc in range(DC):
                    nc.sync.dma_start(x_sb[:, dc, 0:TBS], xT[:, dc, 0:TBS])
                for g in range(FC // fcg):
                    flo, fhi = g * fcg * P, (g + 1) * fcg * P
                    for dc in range(DC):
                        nc.sync.dma_start(w0_sb[:, dc, flo:fhi],
                                          w0T[:, dc, flo:fhi])
                    nc.sync.dma_start(w3_sb[:, g * fcg:(g + 1) * fcg],
                                      w3T[:, g * fcg:(g + 1) * fcg])
                for tb in range(1, NTBS):
                    for dc in range(DC):
                        nc.sync.dma_start(
                            x_sb[:, dc, tb * TBS:(tb + 1) * TBS],
                            xT[:, dc, tb * TBS:(tb + 1) * TBS])

            for tb in range(NTBS):
                o_ps = [
                    opsum.tile([P, DW], f32, name=f"o_ps_{tb}_{i}", tag="o_ps")
                    for i in range(n_ops)
                ]

                def emit_o(fc, g_sb):
                    for ts in range(NTS):
                        for dc2 in range(2):
                            nc.tensor.matmul(
                                o_ps[ts * 2 + dc2][:],
                                g_sb[:, ts * P:(ts + 1) * P],
                                w3_sb[:, fc, dc2 * DW:(dc2 + 1) * DW],
                                start=(fc == 0),
                                stop=(fc == FC - 1),
                            )

                pending = None
                for fc in range(FC):
                    h_ps = hpsum.tile([P, TBS], f32, name=f"h_ps_{tb}_{fc}", tag="h_ps")
                    for dc in range(DC):
                        nc.tensor.matmul(
                            h_ps[:],
                            w0_sb[:, dc, fc * P:(fc + 1) * P],
                            x_sb[:, dc, tb * TBS:(tb + 1) * TBS],
                            start=(dc == 0),
                            stop=(dc == DC - 1),
                        )
                    g_sb = gpool.tile([P, TBS], bf16, name=f"g_{tb}_{fc}", tag="g")
                    nc.scalar.activation(g_sb[:], h_ps[:], AFT.Gelu)
                    if not pipeline_o:
                        emit_o(fc, g_sb)
                    else:
                        if pending is not None:
                            emit_o(*pending)
                        pending = (fc, g_sb)
                if pending is not None:
                    emit_o(*pending)

                for ts in range(NTS):
                    for dc2 in range(2):
                        o_sb = opool.tile([P, DW], f32, name=f"o_sb_{tb}_{ts}_{dc2}",
                                          tag="o_sb")
                        nc.vector.tensor_copy(o_sb[:], o_ps[ts * 2 + dc2][:])
                        nc.sync.dma_start(
                            out[tb * TBS + ts * P: tb * TBS + (ts + 1) * P,
                                dc2 * DW:(dc2 + 1) * DW],
                            o_sb[:],
                        )

    nc.compile()
    return nc


def _build_nc_v2(
    g_extra=0,           # extra gelu-tile slots beyond FC (lookahead into next block)
    h_bufs=3,
    o_ps_bufs=2,
    o_sb_bufs=3,
    x_bufs=2,
    fcg=4,               # fc per w0/w3 DMA group
    x_coarse=True,       # one DMA per x block vs per-dc
    w0_coarse=False,     # one DMA per w0 f-group vs per-dc
    warmup_mms=8,        # scratch matmuls issued before the real work so the
                         # PE rides out the HAM cold-clock window during the
                         # initial DMA wait instead of during real matmuls
):
    """TB=512 two-phase variant: per 512-token block, phase A runs GEMM1+GELU
    for all 32 f-chunks (g tiles [128,512] bf16 stay in SBUF), phase B runs
    GEMM2 as 8 sequential PSUM accumulation groups (one [t=128, d=512] output
    tile each, contraction over all 32 f-chunks). x is streamed per-block
    instead of fully resident to stay under the SBUF cap."""
    import sys
    if "/opt/trn_rl_repo" not in sys.path:
        sys.path.insert(0, "/opt/trn_rl_repo")
    import concourse.tile as tile
    import concourse.mybir as mybir
    from concourse import bacc

    bf16 = mybir.dt.bfloat16
    f32 = mybir.dt.float32
    AFT = mybir.ActivationFunctionType

    TBS = 512
    NTBS = T // TBS      # 4
    NTS = TBS // P       # 4

    G = FC // fcg        # w0 DMA groups
    FW = fcg * P         # f elements per group (512)

    nc = bacc.Bacc(
        "TRN2",
        target_bir_lowering=False,
        debug=False,
        enable_asserts=True,
        num_devices=N_CORES,
    )

    # DRAM layouts are grouped so every load has long (8KB) contiguous
    # per-partition runs: xT by token-block, w0T by f-group.
    xT = nc.dram_tensor("xT", [P, NTBS, DC, TBS], bf16, kind="ExternalInput").ap()
    w0T = nc.dram_tensor("w0T", [P, G, DC, FW], bf16, kind="ExternalInput").ap()
    w3T = nc.dram_tensor("w3T", [P, FC, D], bf16, kind="ExternalInput").ap()
    out = nc.dram_tensor("out", [T, D], f32, kind="ExternalOutput").ap()

    with tile.TileContext(nc) as tc:
        with (
            tc.tile_pool(name="weights", bufs=1) as wpool,
            tc.tile_pool(name="xin", bufs=x_bufs) as xpool,
            tc.tile_pool(name="gelu", bufs=FC + g_extra) as gpool,
            tc.tile_pool(name="ostage", bufs=o_sb_bufs) as opool,
            tc.tile_pool(name="hps", bufs=h_bufs, space="PSUM") as hpsum,
            tc.tile_pool(name="ops", bufs=o_ps_bufs, space="PSUM") as opsum,
        ):
            # w0 SBUF mirrors the grouped DRAM layout; GEMM1 slices
            # [:, fc//fcg, dc, (fc%fcg)*P : +P].
            w0_sb = wpool.tile([P, G, DC, FW], bf16, name="w0_sb", tag="w0_sb")
            w3_sb = wpool.tile([P, FC, D], bf16, name="w3_sb", tag="w3_sb")

            x_tiles = {}
            def load_x(tb):
                xt = xpool.tile([P, DC, TBS], bf16, name=f"x_{tb}", tag="x")
                if x_coarse:
                    nc.sync.dma_start(xt[:], xT[:, tb])
                else:
                    for dc in range(DC):
                        nc.sync.dma_start(xt[:, dc], xT[:, tb, dc])
                x_tiles[tb] = xt

            if warmup_mms:
                with (
                    tc.tile_pool(name="warm", bufs=1) as warmpool,
                    tc.tile_pool(name="warmps", bufs=1, space="PSUM") as warmpsum,
                ):
                    wsrc = warmpool.tile([P, DW], bf16, name="wsrc", tag="wsrc")
                    wps = warmpsum.tile([P, DW], f32, name="wps", tag="wps")
                    nc.gpsimd.memset(wsrc[:], 0.0)
                    for i in range(warmup_mms):
                        nc.tensor.matmul(wps[:], wsrc[:, :P], wsrc[:],
                                         start=(i == 0), stop=(i == warmup_mms - 1))

            # critical prefix: x[tb0], then w0/w3 by f-group in consumption order
            load_x(0)
            for g in range(G):
                if w0_coarse:
                    nc.sync.dma_start(w0_sb[:, g], w0T[:, g])
                else:
                    for dc in range(DC):
                        nc.sync.dma_start(w0_sb[:, g, dc], w0T[:, g, dc])
                nc.sync.dma_start(w3_sb[:, g * fcg:(g + 1) * fcg],
                                  w3T[:, g * fcg:(g + 1) * fcg])

            for tb in range(NTBS):
                if tb + 1 < NTBS:
                    load_x(tb + 1)
                xt = x_tiles.pop(tb)
                # phase A: GEMM1 + GELU for all fc
                g_tiles = []
                for fc in range(FC):
                    h_ps = hpsum.tile([P, TBS], f32, name=f"h_{tb}_{fc}", tag="h_ps")
                    for dc in range(DC):
                        j = fc % fcg
                        nc.tensor.matmul(
                            h_ps[:],
                            w0_sb[:, fc // fcg, dc, j * P:(j + 1) * P],
                            xt[:, dc],
                            start=(dc == 0),
                            stop=(dc == DC - 1),
                        )
                    g_sb = gpool.tile([P, TBS], bf16, name=f"g_{tb}_{fc}", tag="g")
                    nc.scalar.activation(g_sb[:], h_ps[:], AFT.Gelu)
                    g_tiles.append(g_sb)
                # phase B: GEMM2, one [t=128, d=512] accumulation group at a time
                for ts in range(NTS):
                    for dc2 in range(2):
                        o_ps = opsum.tile([P, DW], f32, name=f"o_{tb}_{ts}_{dc2}",
                                          tag="o_ps")
                        for fc in range(FC):
                            nc.tensor.matmul(
                                o_ps[:],
                                g_tiles[fc][:, ts * P:(ts + 1) * P],
                                w3_sb[:, fc, dc2 * DW:(dc2 + 1) * DW],
                                start=(fc == 0),
                                stop=(fc == FC - 1),
                            )
                        o_sb = opool.tile([P, DW], f32, name=f"os_{tb}_{ts}_{dc2}",
                                          tag="o_sb")
                        nc.vector.tensor_copy(o_sb[:], o_ps[:])
                        nc.sync.dma_start(
                            out[tb * TBS + ts * P: tb * TBS + (ts + 1) * P,
                                dc2 * DW:(dc2 + 1) * DW],
                            o_sb[:],
                        )

    nc.compile()
    return nc


def _build_nc_v3(
    g32_bufs=4,
    h_bufs=3,
    o_ps_bufs=2,
    o_sb_bufs=3,
    x_bufs=4,
    warmup_mms=8,
    x_res_pairs=4,     # d-pairs (of 4) that get the x_lo@w0_hi product
    g_res_fpairs=16,   # f-pairs (of 16) that get the g_lo@w3_hi product
    w3_res_fpairs=10,  # f-pairs (of 16) that get the g_hi@w3_lo product
):
    """fp8e4 DoubleRow variant: every operand is represented as two fp8
    digits (hi = fp8(v*s), lo = fp8(v*s - hi)) sharing one scale s per
    tensor, and each 256-wide contraction pair is computed with 3 DoubleRow
    products (hi*hi, lo*hi, hi*lo) at 0.5 cycles/row — 12 products per
    output tile vs bf16's 16 matmuls.

    Scales: x*8, w0*256 -> h_psum = h*2048, GELU applied with input scale
    1/2048; g digits unscaled (s=1); w3*256 -> out_psum = out*256, final
    copy applies 1/256.
    """
    import sys
    if "/opt/trn_rl_repo" not in sys.path:
        sys.path.insert(0, "/opt/trn_rl_repo")
    import concourse.tile as tile
    import concourse.mybir as mybir
    from concourse import bacc

    fp8 = mybir.dt.float8e4
    f32 = mybir.dt.float32
    AFT = mybir.ActivationFunctionType
    DR = mybir.MatmulPerfMode.DoubleRow
    SUB = mybir.AluOpType.subtract

    TBS = 512
    NTBS = T // TBS      # 4
    NTS = TBS // P       # 4
    DPAIR = D // 256     # 4  (256-wide contraction pairs for GEMM1)
    FPAIR = F // 256     # 16 (for GEMM2)
    G1 = 16              # w0 DMA f-groups (small groups -> short critical prefix)
    FW = F // G1         # 256 f per group
    JG = FW // P         # 2 f-chunks per group

    nc = bacc.Bacc(
        "TRN2",
        target_bir_lowering=False,
        debug=False,
        enable_asserts=True,
        num_devices=N_CORES,
    )

    xh_d = nc.dram_tensor("xh", [P, NTBS, DPAIR, 2, TBS], fp8, kind="ExternalInput").ap()
    xl_d = nc.dram_tensor("xl", [P, NTBS, DPAIR, 2, TBS], fp8, kind="ExternalInput").ap()
    w0h_d = nc.dram_tensor("w0h", [P, G1, DPAIR, 2, FW], fp8, kind="ExternalInput").ap()
    w0l_d = nc.dram_tensor("w0l", [P, G1, DPAIR, 2, FW], fp8, kind="ExternalInput").ap()
    w3h_d = nc.dram_tensor("w3h", [P, FPAIR, 2, D], fp8, kind="ExternalInput").ap()
    w3l_d = nc.dram_tensor("w3l", [P, FPAIR, 2, D], fp8, kind="ExternalInput").ap()
    out = nc.dram_tensor("out", [T, D], f32, kind="ExternalOutput").ap()

    with tile.TileContext(nc) as tc:
        with (
            tc.tile_pool(name="weights", bufs=1) as wpool,
            tc.tile_pool(name="xin", bufs=x_bufs) as xpool,
            tc.tile_pool(name="g32", bufs=g32_bufs) as gpool,
            tc.tile_pool(name="ostage", bufs=o_sb_bufs) as opool,
            tc.tile_pool(name="hps", bufs=h_bufs, space="PSUM") as hpsum,
            tc.tile_pool(name="ops", bufs=o_ps_bufs, space="PSUM") as opsum,
        ):
            w0_sb = [wpool.tile([P, G1, DPAIR, 2, FW], fp8, name=f"w0{d}_sb",
                                tag=f"w0{d}") for d in "hl"]
            w3_sb = [wpool.tile([P, FPAIR, 2, D], fp8, name=f"w3{d}_sb",
                                tag=f"w3{d}") for d in "hl"]
            g_sb = [wpool.tile([P, FC, TBS], fp8, name=f"g{d}_sb",
                               tag=f"g{d}") for d in "hl"]

            x_tiles = {}
            def load_x(tb, digits=(0, 1)):
                xt = x_tiles.setdefault(tb, [None, None])
                for dgt in digits:
                    t = xpool.tile([P, DPAIR, 2, TBS], fp8,
                                   name=f"x{'hl'[dgt]}_{tb}", tag=f"x{'hl'[dgt]}")
                    nc.sync.dma_start(t[:], (xh_d, xl_d)[dgt][:, tb])
                    xt[dgt] = t

            if warmup_mms:
                with (
                    tc.tile_pool(name="warm", bufs=1) as warmpool,
                    tc.tile_pool(name="warmps", bufs=1, space="PSUM") as warmpsum,
                ):
                    wsrc = warmpool.tile([P, DW], mybir.dt.bfloat16, name="wsrc",
                                         tag="wsrc")
                    wps = warmpsum.tile([P, DW], f32, name="wps", tag="wps")
                    nc.vector.memset(wsrc[:], 0.0)
                    for i in range(warmup_mms):
                        nc.tensor.matmul(wps[:], wsrc[:, :P], wsrc[:],
                                         start=(i == 0), stop=(i == warmup_mms - 1))

            # critical prefix in first-consumption order (products within an
            # fc run digit-major hh, lh, hl): xh(tb0), w0h g0, xl, w0l g0,
            # then remaining w0 groups, then w3 groups + remaining x blocks.
            load_x(0, digits=(0,))
            nc.sync.dma_start(w0_sb[0][:, 0], w0h_d[:, 0])
            load_x(0, digits=(1,))
            nc.sync.dma_start(w0_sb[1][:, 0], w0l_d[:, 0])
            # stream remaining w0 groups just-ahead of their phase-A use,
            # filling the DMA slack between them with w3 fpair chunks so all
            # of w3 lands before phase B(tb0) consumes its last fpairs.
            w3q = [(fp, dgt) for fp in range(FPAIR) for dgt in range(2)]
            w3i = 0
            for g in range(1, G1):
                nc.sync.dma_start(w0_sb[0][:, g], w0h_d[:, g])
                nc.sync.dma_start(w0_sb[1][:, g], w0l_d[:, g])
                while w3i < min(2 * g, len(w3q)):
                    fp, dgt = w3q[w3i]
                    nc.sync.dma_start(w3_sb[dgt][:, fp:fp + 1],
                                      (w3h_d, w3l_d)[dgt][:, fp:fp + 1])
                    w3i += 1
            while w3i < len(w3q):
                fp, dgt = w3q[w3i]
                nc.sync.dma_start(w3_sb[dgt][:, fp:fp + 1],
                                  (w3h_d, w3l_d)[dgt][:, fp:fp + 1])
                w3i += 1
            for tb in range(1, NTBS):
                load_x(tb)

            for tb in range(NTBS):
                xt = x_tiles.pop(tb)
                # phase A: GEMM1 (DoubleRow, 12 products per fc) + GELU + digits
                for fc in range(FC):
                    g1, j = fc // JG, fc % JG
                    h_ps = hpsum.tile([P, TBS], f32, name=f"h_{tb}_{fc}", tag="h_ps")
                    prods = ([(p, 0, 0) for p in range(DPAIR)]
                             + [(p, 1, 0) for p in range(x_res_pairs)]
                             + [(p, 0, 1) for p in range(DPAIR)])
                    for i, (pair, xa, wb) in enumerate(prods):
                        nc.tensor.matmul(
                            h_ps[:],
                            w0_sb[wb][:, g1, pair, :, j * P:(j + 1) * P],
                            xt[xa][:, pair],
                            start=(i == 0),
                            stop=(i == len(prods) - 1),
                            perf_mode=DR,
                        )
                    g32 = gpool.tile([P, TBS], f32, name=f"g32_{tb}_{fc}", tag="g32")
                    nc.scalar.activation(g32[:], h_ps[:], AFT.Gelu,
                                         scale=1.0 / 2048.0)
                    nc.vector.tensor_copy(g_sb[0][:, fc], g32[:])
                    nc.vector.tensor_tensor(g_sb[1][:, fc], g32[:],
                                            g_sb[0][:, fc], SUB)
                # phase B: GEMM2 (DoubleRow over 16 f-pairs x 3 products)
                for ts in range(NTS):
                    for dc2 in range(2):
                        o_ps = opsum.tile([P, DW], f32, name=f"o_{tb}_{ts}_{dc2}",
                                          tag="o_ps")
                        prods = []
                        for fp in range(FPAIR):
                            prods.append((fp, 0, 0))
                            if fp < g_res_fpairs:
                                prods.append((fp, 1, 0))
                            if fp < w3_res_fpairs:
                                prods.append((fp, 0, 1))
                        for i, (fp, ga, wb) in enumerate(prods):
                            nc.tensor.matmul(
                                o_ps[:],
                                g_sb[ga][:, 2 * fp:2 * fp + 2,
                                         ts * P:(ts + 1) * P],
                                w3_sb[wb][:, fp, :, dc2 * DW:(dc2 + 1) * DW],
                                start=(i == 0),
                                stop=(i == len(prods) - 1),
                                perf_mode=DR,
                            )
                        o_sb = opool.tile([P, DW], f32, name=f"os_{tb}_{ts}_{dc2}",
                                          tag="o_sb")
                        nc.scalar.activation(o_sb[:], o_ps[:], AFT.Copy,
                                             scale=1.0 / 256.0)
                        nc.sync.dma_start(
                            out[tb * TBS + ts * P: tb * TBS + (ts + 1) * P,
                                dc2 * DW:(dc2 + 1) * DW],
                            o_sb[:],
                        )

    nc.compile()
    return nc


def _get_nc():
    if "nc" not in _cache:
        _cache["nc"] = _build_nc_v3()
    return _cache["nc"]


def _make_cached_fn(nc):
    """Build a reusable jitted 8-core executable around bass2jax's bass_exec
    primitive (the same lowering run_bass_kernel_spmd uses under axon), so
    repeat kernel() calls skip retrace/relower."""
    import jax
    import numpy as np
    from jax.sharding import Mesh, PartitionSpec
    try:
        from jax.experimental.shard_map import shard_map
    except ImportError:
        from jax.shard_map import shard_map
    import concourse.mybir as mybir
    from concourse.bass2jax import (_bass_exec_p, install_neuronx_cc_hook,
                                    partition_id_tensor)

    install_neuronx_cc_hook()
    partition_name = nc.partition_id_tensor.name if nc.partition_id_tensor else None
    in_names, out_names, out_avals, zero_shapes = [], [], [], []
    for alloc in nc.m.functions[0].allocations:
        if not isinstance(alloc, mybir.MemoryLocationSet):
            continue
        name = alloc.memorylocations[0].name
        if alloc.kind == "ExternalInput":
            if name != partition_name:
                in_names.append(name)
        elif alloc.kind == "ExternalOutput":
            out_names.append(name)
            shape = tuple(alloc.tensor_shape)
            dtype = mybir.dt.np(alloc.dtype)
            out_avals.append(jax.core.ShapedArray(shape, dtype))
            zero_shapes.append((shape, dtype))
    n_params = len(in_names)
    all_in_names = list(in_names) + list(out_names)
    if partition_name is not None:
        all_in_names.append(partition_name)

    def _body(*args):
        ins = list(args[:n_params])
        outs = list(args[n_params:])
        extra = [partition_id_tensor()] if partition_name is not None else []
        return tuple(_bass_exec_p.bind(
            *ins, *outs, *extra,
            out_avals=tuple(out_avals),
            in_names=tuple(all_in_names),
            out_names=tuple(out_names),
            lowering_input_output_aliases=(),
            sim_require_finite=True,
            sim_require_nnan=True,
            nc=nc,
        ))

    devices = jax.devices()[:N_CORES]
    mesh = Mesh(np.asarray(devices), ("core",))
    fn = jax.jit(
        shard_map(_body, mesh=mesh,
                  in_specs=(PartitionSpec("core"),) * (n_params + len(out_names)),
                  out_specs=(PartitionSpec("core"),) * len(out_names),
                  check_rep=False),
        keep_unused=True)

    def run(in_maps):
        concat_in = [np.concatenate([np.asarray(m[n]) for m in in_maps], axis=0)
                     for n in in_names]
        concat_zeros = [np.zeros((N_CORES * s[0], *s[1:]), dt)
                        for s, dt in zero_shapes]
        outs = fn(*concat_in, *concat_zeros)
        return [
            {name: np.asarray(outs[i]).reshape(N_CORES, *out_avals[i].shape)[c]
             for i, name in enumerate(out_names)}
            for c in range(N_CORES)
        ]

    return run


def kernel(**inputs):
    import os
    import sys
    if "/opt/trn_rl_repo" not in sys.path:
        sys.path.insert(0, "/opt/trn_rl_repo")
    from concourse import bass_utils

    output_tensor = np.asarray(inputs["output_tensor"], dtype=np.float32)  # [1, 8]
    x = np.asarray(inputs["inputs"], dtype=np.float32)   # [1, 8, 2048, 1024]
    w0 = np.asarray(inputs["w0"], dtype=np.float32)      # [8, 4096, 1024]
    w3 = np.asarray(inputs["w3"], dtype=np.float32)      # [8, 1024, 4096]

    fp8 = ml_dtypes.float8_e4m3
    TBS, NTBS = 512, T // 512
    DPAIR, FPAIR = D // 256, F // 256
    G1 = 16
    FW = F // G1

    def split2(a):
        """hi = fp8(a), lo = fp8(a - hi): 2-digit fp8 representation."""
        hi = a.astype(fp8)
        lo = (a - hi.astype(np.float32)).astype(fp8)
        return hi, lo

    def prep_expert(e):
        # Contraction-major layouts with the 256-wide DoubleRow pair split:
        # contraction index c = pair*256 + slot*128 + ki (ki = partition).
        #   x  [128 ki, 4 tb, 4 pair, 2 slot, 512 t]   (scale 8)
        #   w0 [128 ki, 8 g, 4 pair, 2 slot, 512 fw]   (scale 256)
        #   w3 [128 ki, 16 fp, 2 slot, 1024 d]         (scale 256)
        xh, xl = split2(x[0, e].T * 8.0)          # [D, T]
        w0h, w0l = split2(w0[e].T * 256.0)        # [D, F]
        w3h, w3l = split2(w3[e].T * 256.0)        # [F, D]

        def xlay(a):
            return np.ascontiguousarray(
                a.reshape(DPAIR, 2, P, NTBS, TBS).transpose(2, 3, 0, 1, 4))

        def w0lay(a):
            return np.ascontiguousarray(
                a.reshape(DPAIR, 2, P, G1, FW).transpose(2, 3, 0, 1, 4))

        def w3lay(a):
            return np.ascontiguousarray(
                a.reshape(FPAIR, 2, P, D).transpose(2, 0, 1, 3))

        return {
            "xh": xlay(xh), "xl": xlay(xl),
            "w0h": w0lay(w0h), "w0l": w0lay(w0l),
            "w3h": w3lay(w3h), "w3l": w3lay(w3l),
        }

    from concurrent.futures import ThreadPoolExecutor
    with ThreadPoolExecutor(max_workers=N_CORES) as pool:
        in_maps = list(pool.map(prep_expert, range(N_CORES)))

    nc = _get_nc()
    results = None
    if "fast_fn" in _cache:
        try:
            results = _cache["fast_fn"](in_maps)
        except Exception:
            results = None
    if results is None:
        try:
            results = bass_utils.run_bass_kernel_spmd(
                nc, in_maps, core_ids=list(range(N_CORES))).results
        except ModuleNotFoundError:
            # trace path requested via env but axon NTFF hook missing
            os.environ["BASS_NEVER_TRACE"] = "1"
            results = bass_utils.run_bass_kernel_spmd(
                nc, in_maps, core_ids=list(range(N_CORES))).results
        try:
            fast = _make_cached_fn(nc)
            fast(in_maps)  # warm: jit trace + XLA/NEFF compile happens here
            _cache["fast_fn"] = fast
        except Exception:
            pass
    out_full = np.stack([results[e]["out"] for e in range(N_CORES)])[None]

    # unpopular experts with zero gating activity produce zeros
    unpop = output_tensor[:, NUM_LOCAL:].sum(axis=0) != 0
    mask = np.concatenate([np.ones(NUM_LOCAL, dtype=bool), unpop])
    out_full = out_full * mask[None, :, None, None].astype(np.float32)
    return out_full.astype(np.float32)



# revision 17
# speedup vs baseline: 1.4030x; 1.0047x over previous
"""Trainium2 Bass kernel for nn_Experts (grouped MoE expert MLP).

Computes, for each of 8 experts e:
    h   = x_e @ w0_e.T          # [2048,1024] @ [1024,4096] -> [2048,4096]
    g   = gelu_exact(h)
    out = g @ w3_e.T            # [2048,4096] @ [4096,1024] -> [2048,1024]
then masks unpopular experts with zero gating activity (output_tensor).

Sharding: expert-parallel, 1 expert per NeuronCore across 8 cores (SPMD —
one compiled NEFF, per-core input data).

Layout strategy: all operands are pre-transposed on the host into
contraction-major ("K-major") layouts so the device kernel needs no
transposes at all:
    xT  [128, 8, 2048]  (d%128, d//128, t)   bf16
    w0T [128, 8, 4096]  (d%128, d//128, f)   bf16
    w3T [128, 32, 1024] (f%128, f//128, d)   bf16
GEMM1 produces hT tiles [f=128, t] in PSUM, GELU moves them to SBUF as bf16,
and those tiles are directly the lhsT operand of GEMM2 (contraction over f),
whose PSUM output [t=128, d] accumulates over all 32 f-chunks and lands in
the natural [t, d] layout of the output.
"""

import numpy as np
import ml_dtypes

T = 2048      # tokens (capacity) per expert
D = 1024      # hidden
F = 4096      # ffn
P = 128       # partitions
TB = 256      # token block (GEMM1 moving free dim)
NTB = T // TB
DC = D // P   # 8 d-chunks (GEMM1 contraction)
FC = F // P   # 32 f-chunks (GEMM2 contraction)
DW = 512      # GEMM2 output free-dim chunk
NUM_LOCAL = 4
N_CORES = 8

_cache = {}


def _build_nc(
    tb_size=TB,          # token block
    x_split=1,           # extra splits of each x d-chunk DMA (along t)
    w0_split=1,          # extra splits of each w0 d-chunk DMA (along f)
    w3_group=1,          # f-chunks per w3 DMA
    g_bufs=4,
    h_bufs=2,
    o_sb_bufs=4,
    dma_scheme="tuned",  # "simple" | "tuned" (critical-prefix-first ordering)
    fcg=4,               # fc per w0/w3 DMA group in tuned scheme
    pipeline_o=True,     # issue GEMM2(fc) after GEMM1(fc+1) to hide gelu latency
):
    import sys
    if "/opt/trn_rl_repo" not in sys.path:
        sys.path.insert(0, "/opt/trn_rl_repo")
    import concourse.bass as bass
    import concourse.tile as tile
    import concourse.mybir as mybir
    from concourse import bacc

    bf16 = mybir.dt.bfloat16
    f32 = mybir.dt.float32
    AFT = mybir.ActivationFunctionType

    TBS = tb_size
    NTBS = T // TBS
    NTS = TBS // P       # t-subchunks per block (GEMM2 lhsT count)
    n_ops = NTS * 2      # out psum tiles per block ([t 128] x [d 512])

    nc = bacc.Bacc(
        "TRN2",
        target_bir_lowering=False,
        debug=False,
        enable_asserts=True,
        num_devices=N_CORES,
    )

    xT = nc.dram_tensor("xT", [P, DC, T], bf16, kind="ExternalInput").ap()
    w0T = nc.dram_tensor("w0T", [P, DC, F], bf16, kind="ExternalInput").ap()
    w3T = nc.dram_tensor("w3T", [P, FC, D], bf16, kind="ExternalInput").ap()
    out = nc.dram_tensor("out", [T, D], f32, kind="ExternalOutput").ap()

    with tile.TileContext(nc) as tc:
        with (
            tc.tile_pool(name="weights", bufs=1) as wpool,
            tc.tile_pool(name="gelu", bufs=g_bufs) as gpool,
            tc.tile_pool(name="ostage", bufs=o_sb_bufs) as opool,
            tc.tile_pool(name="hps", bufs=h_bufs, space="PSUM") as hpsum,
            tc.tile_pool(name="ops", bufs=n_ops, space="PSUM") as opsum,
        ):
            x_sb = wpool.tile([P, DC, T], bf16, name="x_sb", tag="x_sb")
            w0_sb = wpool.tile([P, DC, F], bf16, name="w0_sb", tag="w0_sb")
            w3_sb = wpool.tile([P, FC, D], bf16, name="w3_sb", tag="w3_sb")

            if dma_scheme == "simple":
                # Load x and w0 first (first h-tile needs ALL d-chunks of
                # both); w3 f-chunks stream in behind.
                for dc in range(DC):
                    for s in range(x_split):
                        w = T // x_split
                        nc.sync.dma_start(x_sb[:, dc, s * w:(s + 1) * w],
                                          xT[:, dc, s * w:(s + 1) * w])
                    for s in range(w0_split):
                        w = F // w0_split
                        nc.sync.dma_start(w0_sb[:, dc, s * w:(s + 1) * w],
                                          w0T[:, dc, s * w:(s + 1) * w])
                for g in range(FC // w3_group):
                    lo, hi = g * w3_group, (g + 1) * w3_group
                    nc.sync.dma_start(w3_sb[:, lo:hi], w3T[:, lo:hi])
            else:
                # Critical-prefix-first: x for tb0, then per-f-group w0 (all
                # d-chunks) and w3 interleaved in the order GEMM1/GEMM2
                # consume them, then the rest of x.
                for dc in range(DC):
                    nc.sync.dma_start(x_sb[:, dc, 0:TBS], xT[:, dc, 0:TBS])
                for g in range(FC // fcg):
                    flo, fhi = g * fcg * P, (g + 1) * fcg * P
                    for dc in range(DC):
                        nc.sync.dma_start(w0_sb[:, dc, flo:fhi],
                                          w0T[:, dc, flo:fhi])
                    nc.sync.dma_start(w3_sb[:, g * fcg:(g + 1) * fcg],
                                      w3T[:, g * fcg:(g + 1) * fcg])
                for tb in range(1, NTBS):
                    for dc in range(DC):
                        nc.sync.dma_start(
                            x_sb[:, dc, tb * TBS:(tb + 1) * TBS],
                            xT[:, dc, tb * TBS:(tb + 1) * TBS])

            for tb in range(NTBS):
                o_ps = [
                    opsum.tile([P, DW], f32, name=f"o_ps_{tb}_{i}", tag="o_ps")
                    for i in range(n_ops)
                ]

                def emit_o(fc, g_sb):
                    for ts in range(NTS):
                        for dc2 in range(2):
                            nc.tensor.matmul(
                                o_ps[ts * 2 + dc2][:],
                                g_sb[:, ts * P:(ts + 1) * P],
                                w3_sb[:, fc, dc2 * DW:(dc2 + 1) * DW],
                                start=(fc == 0),
                                stop=(fc == FC - 1),
                            )

                pending = None
                for fc in range(FC):
                    h_ps = hpsum.tile([P, TBS], f32, name=f"h_ps_{tb}_{fc}", tag="h_ps")
                    for dc in range(DC):
                        nc.tensor.matmul(
                            h_ps[:],
                            w0_sb[:, dc, fc * P:(fc + 1) * P],
                            x_sb[:, dc, tb * TBS:(tb + 1) * TBS],
                            start=(dc == 0),
                            stop=(dc == DC - 1),
                        )
                    g_sb = gpool.tile([P, TBS], bf16, name=f"g_{tb}_{fc}", tag="g")
                    nc.scalar.activation(g_sb[:], h_ps[:], AFT.Gelu)
                    if not pipeline_o:
                        emit_o(fc, g_sb)
                    else:
                        if pending is not None:
                            emit_o(*pending)
                        pending = (fc, g_sb)
                if pending is not None:
                    emit_o(*pending)

                for ts in range(NTS):
                    for dc2 in range(2):
                        o_sb = opool.tile([P, DW], f32, name=f"o_sb_{tb}_{ts}_{dc2}",
                                          tag="o_sb")
                        nc.vector.tensor_copy(o_sb[:], o_ps[ts * 2 + dc2][:])
                        nc.sync.dma_start(
                            out[tb * TBS + ts * P: tb * TBS + (ts + 1) * P,
                                dc2 * DW:(dc2 + 1) * DW],
                            o_sb[:],
                        )

    nc.compile()
    return nc


def _build_nc_v2(
    g_extra=0,           # extra gelu-tile slots beyond FC (lookahead into next block)
    h_bufs=3,
    o_ps_bufs=2,
    o_sb_bufs=3,
    x_bufs=2,
    fcg=4,               # fc per w0/w3 DMA group
    x_coarse=True,       # one DMA per x block vs per-dc
    w0_coarse=False,     # one DMA per w0 f-group vs per-dc
    warmup_mms=8,        # scratch matmuls issued before the real work so the
                         # PE rides out the HAM cold-clock window during the
                         # initial DMA wait instead of during real matmuls
):
    """TB=512 two-phase variant: per 512-token block, phase A runs GEMM1+GELU
    for all 32 f-chunks (g tiles [128,512] bf16 stay in SBUF), phase B runs
    GEMM2 as 8 sequential PSUM accumulation groups (one [t=128, d=512] output
    tile each, contraction over all 32 f-chunks). x is streamed per-block
    instead of fully resident to stay under the SBUF cap."""
    import sys
    if "/opt/trn_rl_repo" not in sys.path:
        sys.path.insert(0, "/opt/trn_rl_repo")
    import concourse.tile as tile
    import concourse.mybir as mybir
    from concourse import bacc

    bf16 = mybir.dt.bfloat16
    f32 = mybir.dt.float32
    AFT = mybir.ActivationFunctionType

    TBS = 512
    NTBS = T // TBS      # 4
    NTS = TBS // P       # 4

    G = FC // fcg        # w0 DMA groups
    FW = fcg * P         # f elements per group (512)

    nc = bacc.Bacc(
        "TRN2",
        target_bir_lowering=False,
        debug=False,
        enable_asserts=True,
        num_devices=N_CORES,
    )

    # DRAM layouts are grouped so every load has long (8KB) contiguous
    # per-partition runs: xT by token-block, w0T by f-group.
    xT = nc.dram_tensor("xT", [P, NTBS, DC, TBS], bf16, kind="ExternalInput").ap()
    w0T = nc.dram_tensor("w0T", [P, G, DC, FW], bf16, kind="ExternalInput").ap()
    w3T = nc.dram_tensor("w3T", [P, FC, D], bf16, kind="ExternalInput").ap()
    out = nc.dram_tensor("out", [T, D], f32, kind="ExternalOutput").ap()

    with tile.TileContext(nc) as tc:
        with (
            tc.tile_pool(name="weights", bufs=1) as wpool,
            tc.tile_pool(name="xin", bufs=x_bufs) as xpool,
            tc.tile_pool(name="gelu", bufs=FC + g_extra) as gpool,
            tc.tile_pool(name="ostage", bufs=o_sb_bufs) as opool,
            tc.tile_pool(name="hps", bufs=h_bufs, space="PSUM") as hpsum,
            tc.tile_pool(name="ops", bufs=o_ps_bufs, space="PSUM") as opsum,
        ):
            # w0 SBUF mirrors the grouped DRAM layout; GEMM1 slices
            # [:, fc//fcg, dc, (fc%fcg)*P : +P].
            w0_sb = wpool.tile([P, G, DC, FW], bf16, name="w0_sb", tag="w0_sb")
            w3_sb = wpool.tile([P, FC, D], bf16, name="w3_sb", tag="w3_sb")

            x_tiles = {}
            def load_x(tb):
                xt = xpool.tile([P, DC, TBS], bf16, name=f"x_{tb}", tag="x")
                if x_coarse:
                    nc.sync.dma_start(xt[:], xT[:, tb])
                else:
                    for dc in range(DC):
                        nc.sync.dma_start(xt[:, dc], xT[:, tb, dc])
                x_tiles[tb] = xt

            if warmup_mms:
                with (
                    tc.tile_pool(name="warm", bufs=1) as warmpool,
                    tc.tile_pool(name="warmps", bufs=1, space="PSUM") as warmpsum,
                ):
                    wsrc = warmpool.tile([P, DW], bf16, name="wsrc", tag="wsrc")
                    wps = warmpsum.tile([P, DW], f32, name="wps", tag="wps")
                    nc.gpsimd.memset(wsrc[:], 0.0)
                    for i in range(warmup_mms):
                        nc.tensor.matmul(wps[:], wsrc[:, :P], wsrc[:],
                                         start=(i == 0), stop=(i == warmup_mms - 1))

            # critical prefix: x[tb0], then w0/w3 by f-group in consumption order
            load_x(0)
            for g in range(G):
                if w0_coarse:
                    nc.sync.dma_start(w0_sb[:, g], w0T[:, g])
                else:
                    for dc in range(DC):
                        nc.sync.dma_start(w0_sb[:, g, dc], w0T[:, g, dc])
                nc.sync.dma_start(w3_sb[:, g * fcg:(g + 1) * fcg],
                                  w3T[:, g * fcg:(g + 1) * fcg])

            for tb in range(NTBS):
                if tb + 1 < NTBS:
                    load_x(tb + 1)
                xt = x_tiles.pop(tb)
                # phase A: GEMM1 + GELU for all fc
                g_tiles = []
                for fc in range(FC):
                    h_ps = hpsum.tile([P, TBS], f32, name=f"h_{tb}_{fc}", tag="h_ps")
                    for dc in range(DC):
                        j = fc % fcg
                        nc.tensor.matmul(
                            h_ps[:],
                            w0_sb[:, fc // fcg, dc, j * P:(j + 1) * P],
                            xt[:, dc],
                            start=(dc == 0),
                            stop=(dc == DC - 1),
                        )
                    g_sb = gpool.tile([P, TBS], bf16, name=f"g_{tb}_{fc}", tag="g")
                    nc.scalar.activation(g_sb[:], h_ps[:], AFT.Gelu)
                    g_tiles.append(g_sb)
                # phase B: GEMM2, one [t=128, d=512] accumulation group at a time
                for ts in range(NTS):
                    for dc2 in range(2):
                        o_ps = opsum.tile([P, DW], f32, name=f"o_{tb}_{ts}_{dc2}",
                                          tag="o_ps")
                        for fc in range(FC):
                            nc.tensor.matmul(
                                o_ps[:],
                                g_tiles[fc][:, ts * P:(ts + 1) * P],
                                w3_sb[:, fc, dc2 * DW:(dc2 + 1) * DW],
                                start=(fc == 0),
                                stop=(fc == FC - 1),
                            )
                        o_sb = opool.tile([P, DW], f32, name=f"os_{tb}_{ts}_{dc2}",
                                          tag="o_sb")
                        nc.vector.tensor_copy(o_sb[:], o_ps[:])
                        nc.sync.dma_start(
                            out[tb * TBS + ts * P: tb * TBS + (ts + 1) * P,
                                dc2 * DW:(dc2 + 1) * DW],
                            o_sb[:],
                        )

    nc.compile()
    return nc


def _build_nc_v3(
    g32_bufs=4,
    h_bufs=3,
    o_ps_bufs=2,
    o_sb_bufs=3,
    x_bufs=4,
    warmup_mms=32,
    x_res_pairs=4,     # d-pairs (of 4) that get the x_lo@w0_hi product
    g_res_fpairs=16,   # f-pairs (of 16) that get the g_lo@w3_hi product
    w3_res_fpairs=10,  # f-pairs (of 16) that get the g_hi@w3_lo product
):
    """fp8e4 DoubleRow variant: every operand is represented as two fp8
    digits (hi = fp8(v*s), lo = fp8(v*s - hi)) sharing one scale s per
    tensor, and each 256-wide contraction pair is computed with 3 DoubleRow
    products (hi*hi, lo*hi, hi*lo) at 0.5 cycles/row — 12 products per
    output tile vs bf16's 16 matmuls.

    Scales: x*8, w0*256 -> h_psum = h*2048, GELU applied with input scale
    1/2048; g digits unscaled (s=1); w3*256 -> out_psum = out*256, final
    copy applies 1/256.
    """
    import sys
    if "/opt/trn_rl_repo" not in sys.path:
        sys.path.insert(0, "/opt/trn_rl_repo")
    import concourse.tile as tile
    import concourse.mybir as mybir
    from concourse import bacc

    fp8 = mybir.dt.float8e4
    f32 = mybir.dt.float32
    AFT = mybir.ActivationFunctionType
    DR = mybir.MatmulPerfMode.DoubleRow
    SUB = mybir.AluOpType.subtract

    TBS = 512
    NTBS = T // TBS      # 4
    NTS = TBS // P       # 4
    DPAIR = D // 256     # 4  (256-wide contraction pairs for GEMM1)
    FPAIR = F // 256     # 16 (for GEMM2)
    G1 = 16              # w0 DMA f-groups (small groups -> short critical prefix)
    FW = F // G1         # 256 f per group
    JG = FW // P         # 2 f-chunks per group

    nc = bacc.Bacc(
        "TRN2",
        target_bir_lowering=False,
        debug=False,
        enable_asserts=True,
        num_devices=N_CORES,
    )

    xh_d = nc.dram_tensor("xh", [P, NTBS, DPAIR, 2, TBS], fp8, kind="ExternalInput").ap()
    xl_d = nc.dram_tensor("xl", [P, NTBS, DPAIR, 2, TBS], fp8, kind="ExternalInput").ap()
    w0h_d = nc.dram_tensor("w0h", [P, G1, DPAIR, 2, FW], fp8, kind="ExternalInput").ap()
    w0l_d = nc.dram_tensor("w0l", [P, G1, DPAIR, 2, FW], fp8, kind="ExternalInput").ap()
    w3h_d = nc.dram_tensor("w3h", [P, FPAIR, 2, D], fp8, kind="ExternalInput").ap()
    w3l_d = nc.dram_tensor("w3l", [P, FPAIR, 2, D], fp8, kind="ExternalInput").ap()
    out = nc.dram_tensor("out", [T, D], f32, kind="ExternalOutput").ap()

    with tile.TileContext(nc) as tc:
        with (
            tc.tile_pool(name="weights", bufs=1) as wpool,
            tc.tile_pool(name="xin", bufs=x_bufs) as xpool,
            tc.tile_pool(name="g32", bufs=g32_bufs) as gpool,
            tc.tile_pool(name="ostage", bufs=o_sb_bufs) as opool,
            tc.tile_pool(name="hps", bufs=h_bufs, space="PSUM") as hpsum,
            tc.tile_pool(name="ops", bufs=o_ps_bufs, space="PSUM") as opsum,
        ):
            w0_sb = [wpool.tile([P, G1, DPAIR, 2, FW], fp8, name=f"w0{d}_sb",
                                tag=f"w0{d}") for d in "hl"]
            w3_sb = [wpool.tile([P, FPAIR, 2, D], fp8, name=f"w3{d}_sb",
                                tag=f"w3{d}") for d in "hl"]
            g_sb = [wpool.tile([P, FC, TBS], fp8, name=f"g{d}_sb",
                               tag=f"g{d}") for d in "hl"]

            x_tiles = {}
            def load_x(tb, digits=(0, 1)):
                xt = x_tiles.setdefault(tb, [None, None])
                for dgt in digits:
                    t = xpool.tile([P, DPAIR, 2, TBS], fp8,
                                   name=f"x{'hl'[dgt]}_{tb}", tag=f"x{'hl'[dgt]}")
                    nc.sync.dma_start(t[:], (xh_d, xl_d)[dgt][:, tb])
                    xt[dgt] = t

            if warmup_mms:
                with (
                    tc.tile_pool(name="warm", bufs=1) as warmpool,
                    tc.tile_pool(name="warmps", bufs=1, space="PSUM") as warmpsum,
                ):
                    wsrc = warmpool.tile([P, P], mybir.dt.bfloat16, name="wsrc",
                                         tag="wsrc")
                    wps = warmpsum.tile([P, P], f32, name="wps", tag="wps")
                    nc.vector.memset(wsrc[:], 0.0)
                    for i in range(warmup_mms):
                        nc.tensor.matmul(wps[:], wsrc[:], wsrc[:],
                                         start=(i == 0), stop=(i == warmup_mms - 1))

            # critical prefix in first-consumption order (products within an
            # fc run digit-major hh, lh, hl): xh(tb0), w0h g0, xl, w0l g0,
            # then remaining w0 groups, then w3 groups + remaining x blocks.
            load_x(0, digits=(0,))
            nc.sync.dma_start(w0_sb[0][:, 0], w0h_d[:, 0])
            load_x(0, digits=(1,))
            nc.sync.dma_start(w0_sb[1][:, 0], w0l_d[:, 0])
            # remaining w0 groups back-to-back (phase A consumes 1 group per
            # ~2.5us vs ~1.5us DMA), then all of w3 in fpair order (needed
            # from phase B(tb0) at ~48us; lands by ~50... the last fpairs are
            # read last), then the remaining x blocks (needed at ~90us+).
            for g in range(1, G1):
                nc.sync.dma_start(w0_sb[0][:, g], w0h_d[:, g])
                nc.sync.dma_start(w0_sb[1][:, g], w0l_d[:, g])
            for fp in range(FPAIR):
                nc.sync.dma_start(w3_sb[0][:, fp:fp + 1], w3h_d[:, fp:fp + 1])
                nc.sync.dma_start(w3_sb[1][:, fp:fp + 1], w3l_d[:, fp:fp + 1])
            for tb in range(1, NTBS):
                load_x(tb)

            for tb in range(NTBS):
                xt = x_tiles.pop(tb)
                # phase A: GEMM1 (DoubleRow, 12 products per fc) + GELU + digits
                for fc in range(FC):
                    g1, j = fc // JG, fc % JG
                    h_ps = hpsum.tile([P, TBS], f32, name=f"h_{tb}_{fc}", tag="h_ps")
                    prods = ([(p, 0, 0) for p in range(DPAIR)]
                             + [(p, 1, 0) for p in range(x_res_pairs)]
                             + [(p, 0, 1) for p in range(DPAIR)])
                    for i, (pair, xa, wb) in enumerate(prods):
                        nc.tensor.matmul(
                            h_ps[:],
                            w0_sb[wb][:, g1, pair, :, j * P:(j + 1) * P],
                            xt[xa][:, pair],
                            start=(i == 0),
                            stop=(i == len(prods) - 1),
                            perf_mode=DR,
                        )
                    g32 = gpool.tile([P, TBS], f32, name=f"g32_{tb}_{fc}", tag="g32")
                    nc.scalar.activation(g32[:], h_ps[:], AFT.Gelu,
                                         scale=1.0 / 2048.0)
                    nc.vector.tensor_copy(g_sb[0][:, fc], g32[:])
                    nc.vector.tensor_tensor(g_sb[1][:, fc], g32[:],
                                            g_sb[0][:, fc], SUB)
                # phase B: GEMM2 (DoubleRow over 16 f-pairs x 3 products)
                for ts in range(NTS):
                    for dc2 in range(2):
                        o_ps = opsum.tile([P, DW], f32, name=f"o_{tb}_{ts}_{dc2}",
                                          tag="o_ps")
                        prods = []
                        for fp in range(FPAIR):
                            prods.append((fp, 0, 0))
                            if fp < g_res_fpairs:
                                prods.append((fp, 1, 0))
                            if fp < w3_res_fpairs:
                                prods.append((fp, 0, 1))
                        for i, (fp, ga, wb) in enumerate(prods):
                            nc.tensor.matmul(
                                o_ps[:],
                                g_sb[ga][:, 2 * fp:2 * fp + 2,
                                         ts * P:(ts + 1) * P],
                                w3_sb[wb][:, fp, :, dc2 * DW:(dc2 + 1) * DW],
                                start=(i == 0),
                                stop=(i == len(prods) - 1),
                                perf_mode=DR,
                            )
                        o_sb = opool.tile([P, DW], f32, name=f"os_{tb}_{ts}_{dc2}",
                                          tag="o_sb")
                        nc.scalar.activation(o_sb[:], o_ps[:], AFT.Copy,
                                             scale=1.0 / 256.0)
                        nc.sync.dma_start(
                            out[tb * TBS + ts * P: tb * TBS + (ts + 1) * P,
                                dc2 * DW:(dc2 + 1) * DW],
                            o_sb[:],
                        )

    nc.compile()
    return nc


def _get_nc():
    if "nc" not in _cache:
        _cache["nc"] = _build_nc_v3()
    return _cache["nc"]


def _make_cached_fn(nc):
    """Build a reusable jitted 8-core executable around bass2jax's bass_exec
    primitive (the same lowering run_bass_kernel_spmd uses under axon), so
    repeat kernel() calls skip retrace/relower."""
    import jax
    import numpy as np
    from jax.sharding import Mesh, PartitionSpec
    try:
        from jax.experimental.shard_map import shard_map
    except ImportError:
        from jax.shard_map import shard_map
    import concourse.mybir as mybir
    from concourse.bass2jax import (_bass_exec_p, install_neuronx_cc_hook,
                                    partition_id_tensor)

    install_neuronx_cc_hook()
    partition_name = nc.partition_id_tensor.name if nc.partition_id_tensor else None
    in_names, out_names, out_avals, zero_shapes = [], [], [], []
    for alloc in nc.m.functions[0].allocations:
        if not isinstance(alloc, mybir.MemoryLocationSet):
            continue
        name = alloc.memorylocations[0].name
        if alloc.kind == "ExternalInput":
            if name != partition_name:
                in_names.append(name)
        elif alloc.kind == "ExternalOutput":
            out_names.append(name)
            shape = tuple(alloc.tensor_shape)
            dtype = mybir.dt.np(alloc.dtype)
            out_avals.append(jax.core.ShapedArray(shape, dtype))
            zero_shapes.append((shape, dtype))
    n_params = len(in_names)
    all_in_names = list(in_names) + list(out_names)
    if partition_name is not None:
        all_in_names.append(partition_name)

    def _body(*args):
        ins = list(args[:n_params])
        outs = list(args[n_params:])
        extra = [partition_id_tensor()] if partition_name is not None else []
        return tuple(_bass_exec_p.bind(
            *ins, *outs, *extra,
            out_avals=tuple(out_avals),
            in_names=tuple(all_in_names),
            out_names=tuple(out_names),
            lowering_input_output_aliases=(),
            sim_require_finite=True,
            sim_require_nnan=True,
            nc=nc,
        ))

    devices = jax.devices()[:N_CORES]
    mesh = Mesh(np.asarray(devices), ("core",))
    fn = jax.jit(
        shard_map(_body, mesh=mesh,
                  in_specs=(PartitionSpec("core"),) * (n_params + len(out_names)),
                  out_specs=(PartitionSpec("core"),) * len(out_names),
                  check_rep=False),
        keep_unused=True)

    def run(in_maps):
        concat_in = [np.concatenate([np.asarray(m[n]) for m in in_maps], axis=0)
                     for n in in_names]
        concat_zeros = [np.zeros((N_CORES * s[0], *s[1:]), dt)
                        for s, dt in zero_shapes]
        outs = fn(*concat_in, *concat_zeros)
        return [
            {name: np.asarray(outs[i]).reshape(N_CORES, *out_avals[i].shape)[c]
             for i, name in enumerate(out_names)}
            for c in range(N_CORES)
        ]

    return run


def kernel(**inputs):
    import os
    import sys
    if "/opt/trn_rl_repo" not in sys.path:
        sys.path.insert(0, "/opt/trn_rl_repo")
    from concourse import bass_utils

    output_tensor = np.asarray(inputs["output_tensor"], dtype=np.float32)  # [1, 8]
    x = np.asarray(inputs["inputs"], dtype=np.float32)   # [1, 8, 2048, 1024]
    w0 = np.asarray(inputs["w0"], dtype=np.float32)      # [8, 4096, 1024]
    w3 = np.asarray(inputs["w3"], dtype=np.float32)      # [8, 1024, 4096]

    fp8 = ml_dtypes.float8_e4m3
    TBS, NTBS = 512, T // 512
    DPAIR, FPAIR = D // 256, F // 256
    G1 = 16
    FW = F // G1

    def split2(a):
        """hi = fp8(a), lo = fp8(a - hi): 2-digit fp8 representation."""
        hi = a.astype(fp8)
        lo = (a - hi.astype(np.float32)).astype(fp8)
        return hi, lo

    def prep_expert(e):
        # Contraction-major layouts with the 256-wide DoubleRow pair split:
        # contraction index c = pair*256 + slot*128 + ki (ki = partition).
        #   x  [128 ki, 4 tb, 4 pair, 2 slot, 512 t]   (scale 8)
        #   w0 [128 ki, 8 g, 4 pair, 2 slot, 512 fw]   (scale 256)
        #   w3 [128 ki, 16 fp, 2 slot, 1024 d]         (scale 256)
        xh, xl = split2(x[0, e].T * 8.0)          # [D, T]
        w0h, w0l = split2(w0[e].T * 256.0)        # [D, F]
        w3h, w3l = split2(w3[e].T * 256.0)        # [F, D]

        def xlay(a):
            return np.ascontiguousarray(
                a.reshape(DPAIR, 2, P, NTBS, TBS).transpose(2, 3, 0, 1, 4))

        def w0lay(a):
            return np.ascontiguousarray(
                a.reshape(DPAIR, 2, P, G1, FW).transpose(2, 3, 0, 1, 4))

        def w3lay(a):
            return np.ascontiguousarray(
                a.reshape(FPAIR, 2, P, D).transpose(2, 0, 1, 3))

        return {
            "xh": xlay(xh), "xl": xlay(xl),
            "w0h": w0lay(w0h), "w0l": w0lay(w0l),
            "w3h": w3lay(w3h), "w3l": w3lay(w3l),
        }

    from concurrent.futures import ThreadPoolExecutor
    with ThreadPoolExecutor(max_workers=N_CORES) as pool:
        in_maps = list(pool.map(prep_expert, range(N_CORES)))

    nc = _get_nc()
    results = None
    if "fast_fn" in _cache:
        try:
            results = _cache["fast_fn"](in_maps)
        except Exception:
            results = None
    if results is None:
        try:
            results = bass_utils.run_bass_kernel_spmd(
                nc, in_maps, core_ids=list(range(N_CORES))).results
        except ModuleNotFoundError:
            # trace path requested via env but axon NTFF hook missing
            os.environ["BASS_NEVER_TRACE"] = "1"
            results = bass_utils.run_bass_kernel_spmd(
                nc, in_maps, core_ids=list(range(N_CORES))).results
        try:
            fast = _make_cached_fn(nc)
            fast(in_maps)  # warm: jit trace + XLA/NEFF compile happens here
            _cache["fast_fn"] = fast
        except Exception:
            pass
    out_full = np.stack([results[e]["out"] for e in range(N_CORES)])[None]

    # unpopular experts with zero gating activity produce zeros
    unpop = output_tensor[:, NUM_LOCAL:].sum(axis=0) != 0
    mask = np.concatenate([np.ones(NUM_LOCAL, dtype=bool), unpop])
    out_full = out_full * mask[None, :, None, None].astype(np.float32)
    return out_full.astype(np.float32)



# revision 26
# speedup vs baseline: 1.4054x; 1.0017x over previous
"""Trainium2 Bass kernel for nn_Experts (grouped MoE expert MLP).

Computes, for each of 8 experts e:
    h   = x_e @ w0_e.T          # [2048,1024] @ [1024,4096] -> [2048,4096]
    g   = gelu_exact(h)
    out = g @ w3_e.T            # [2048,4096] @ [4096,1024] -> [2048,1024]
then masks unpopular experts with zero gating activity (output_tensor).

Sharding: expert-parallel, 1 expert per NeuronCore across 8 cores (SPMD —
one compiled NEFF, per-core input data).

Layout strategy: all operands are pre-transposed on the host into
contraction-major ("K-major") layouts so the device kernel needs no
transposes at all:
    xT  [128, 8, 2048]  (d%128, d//128, t)   bf16
    w0T [128, 8, 4096]  (d%128, d//128, f)   bf16
    w3T [128, 32, 1024] (f%128, f//128, d)   bf16
GEMM1 produces hT tiles [f=128, t] in PSUM, GELU moves them to SBUF as bf16,
and those tiles are directly the lhsT operand of GEMM2 (contraction over f),
whose PSUM output [t=128, d] accumulates over all 32 f-chunks and lands in
the natural [t, d] layout of the output.
"""

import numpy as np
import ml_dtypes

T = 2048      # tokens (capacity) per expert
D = 1024      # hidden
F = 4096      # ffn
P = 128       # partitions
TB = 256      # token block (GEMM1 moving free dim)
NTB = T // TB
DC = D // P   # 8 d-chunks (GEMM1 contraction)
FC = F // P   # 32 f-chunks (GEMM2 contraction)
DW = 512      # GEMM2 output free-dim chunk
NUM_LOCAL = 4
N_CORES = 8

_cache = {}


def _build_nc(
    tb_size=TB,          # token block
    x_split=1,           # extra splits of each x d-chunk DMA (along t)
    w0_split=1,          # extra splits of each w0 d-chunk DMA (along f)
    w3_group=1,          # f-chunks per w3 DMA
    g_bufs=4,
    h_bufs=2,
    o_sb_bufs=4,
    dma_scheme="tuned",  # "simple" | "tuned" (critical-prefix-first ordering)
    fcg=4,               # fc per w0/w3 DMA group in tuned scheme
    pipeline_o=True,     # issue GEMM2(fc) after GEMM1(fc+1) to hide gelu latency
):
    import sys
    if "/opt/trn_rl_repo" not in sys.path:
        sys.path.insert(0, "/opt/trn_rl_repo")
    import concourse.bass as bass
    import concourse.tile as tile
    import concourse.mybir as mybir
    from concourse import bacc

    bf16 = mybir.dt.bfloat16
    f32 = mybir.dt.float32
    AFT = mybir.ActivationFunctionType

    TBS = tb_size
    NTBS = T // TBS
    NTS = TBS // P       # t-subchunks per block (GEMM2 lhsT count)
    n_ops = NTS * 2      # out psum tiles per block ([t 128] x [d 512])

    nc = bacc.Bacc(
        "TRN2",
        target_bir_lowering=False,
        debug=False,
        enable_asserts=True,
        num_devices=N_CORES,
    )

    xT = nc.dram_tensor("xT", [P, DC, T], bf16, kind="ExternalInput").ap()
    w0T = nc.dram_tensor("w0T", [P, DC, F], bf16, kind="ExternalInput").ap()
    w3T = nc.dram_tensor("w3T", [P, FC, D], bf16, kind="ExternalInput").ap()
    out = nc.dram_tensor("out", [T, D], f32, kind="ExternalOutput").ap()

    with tile.TileContext(nc) as tc:
        with (
            tc.tile_pool(name="weights", bufs=1) as wpool,
            tc.tile_pool(name="gelu", bufs=g_bufs) as gpool,
            tc.tile_pool(name="ostage", bufs=o_sb_bufs) as opool,
            tc.tile_pool(name="hps", bufs=h_bufs, space="PSUM") as hpsum,
            tc.tile_pool(name="ops", bufs=n_ops, space="PSUM") as opsum,
        ):
            x_sb = wpool.tile([P, DC, T], bf16, name="x_sb", tag="x_sb")
            w0_sb = wpool.tile([P, DC, F], bf16, name="w0_sb", tag="w0_sb")
            w3_sb = wpool.tile([P, FC, D], bf16, name="w3_sb", tag="w3_sb")

            if dma_scheme == "simple":
                # Load x and w0 first (first h-tile needs ALL d-chunks of
                # both); w3 f-chunks stream in behind.
                for dc in range(DC):
                    for s in range(x_split):
                        w = T // x_split
                        nc.sync.dma_start(x_sb[:, dc, s * w:(s + 1) * w],
                                          xT[:, dc, s * w:(s + 1) * w])
                    for s in range(w0_split):
                        w = F // w0_split
                        nc.sync.dma_start(w0_sb[:, dc, s * w:(s + 1) * w],
                                          w0T[:, dc, s * w:(s + 1) * w])
                for g in range(FC // w3_group):
                    lo, hi = g * w3_group, (g + 1) * w3_group
                    nc.sync.dma_start(w3_sb[:, lo:hi], w3T[:, lo:hi])
            else:
                # Critical-prefix-first: x for tb0, then per-f-group w0 (all
                # d-chunks) and w3 interleaved in the order GEMM1/GEMM2
                # consume them, then the rest of x.
                for dc in range(DC):
                    nc.sync.dma_start(x_sb[:, dc, 0:TBS], xT[:, dc, 0:TBS])
                for g in range(FC // fcg):
                    flo, fhi = g * fcg * P, (g + 1) * fcg * P
                    for dc in range(DC):
                        nc.sync.dma_start(w0_sb[:, dc, flo:fhi],
                                          w0T[:, dc, flo:fhi])
                    nc.sync.dma_start(w3_sb[:, g * fcg:(g + 1) * fcg],
                                      w3T[:, g * fcg:(g + 1) * fcg])
                for tb in range(1, NTBS):
                    for dc in range(DC):
                        nc.sync.dma_start(
                            x_sb[:, dc, tb * TBS:(tb + 1) * TBS],
                            xT[:, dc, tb * TBS:(tb + 1) * TBS])

            for tb in range(NTBS):
                o_ps = [
                    opsum.tile([P, DW], f32, name=f"o_ps_{tb}_{i}", tag="o_ps")
                    for i in range(n_ops)
                ]

                def emit_o(fc, g_sb):
                    for ts in range(NTS):
                        for dc2 in range(2):
                            nc.tensor.matmul(
                                o_ps[ts * 2 + dc2][:],
                                g_sb[:, ts * P:(ts + 1) * P],
                                w3_sb[:, fc, dc2 * DW:(dc2 + 1) * DW],
                                start=(fc == 0),
                                stop=(fc == FC - 1),
                            )

                pending = None
                for fc in range(FC):
                    h_ps = hpsum.tile([P, TBS], f32, name=f"h_ps_{tb}_{fc}", tag="h_ps")
                    for dc in range(DC):
                        nc.tensor.matmul(
                            h_ps[:],
                            w0_sb[:, dc, fc * P:(fc + 1) * P],
                            x_sb[:, dc, tb * TBS:(tb + 1) * TBS],
                            start=(dc == 0),
                            stop=(dc == DC - 1),
                        )
                    g_sb = gpool.tile([P, TBS], bf16, name=f"g_{tb}_{fc}", tag="g")
                    nc.scalar.activation(g_sb[:], h_ps[:], AFT.Gelu)
                    if not pipeline_o:
                        emit_o(fc, g_sb)
                    else:
                        if pending is not None:
                            emit_o(*pending)
                        pending = (fc, g_sb)
                if pending is not None:
                    emit_o(*pending)

                for ts in range(NTS):
                    for dc2 in range(2):
                        o_sb = opool.tile([P, DW], f32, name=f"o_sb_{tb}_{ts}_{dc2}",
                                          tag="o_sb")
                        nc.vector.tensor_copy(o_sb[:], o_ps[ts * 2 + dc2][:])
                        nc.sync.dma_start(
                            out[tb * TBS + ts * P: tb * TBS + (ts + 1) * P,
                                dc2 * DW:(dc2 + 1) * DW],
                            o_sb[:],
                        )

    nc.compile()
    return nc


def _build_nc_v2(
    g_extra=0,           # extra gelu-tile slots beyond FC (lookahead into next block)
    h_bufs=3,
    o_ps_bufs=2,
    o_sb_bufs=3,
    x_bufs=2,
    fcg=4,               # fc per w0/w3 DMA group
    x_coarse=True,       # one DMA per x block vs per-dc
    w0_coarse=False,     # one DMA per w0 f-group vs per-dc
    warmup_mms=8,        # scratch matmuls issued before the real work so the
                         # PE rides out the HAM cold-clock window during the
                         # initial DMA wait instead of during real matmuls
):
    """TB=512 two-phase variant: per 512-token block, phase A runs GEMM1+GELU
    for all 32 f-chunks (g tiles [128,512] bf16 stay in SBUF), phase B runs
    GEMM2 as 8 sequential PSUM accumulation groups (one [t=128, d=512] output
    tile each, contraction over all 32 f-chunks). x is streamed per-block
    instead of fully resident to stay under the SBUF cap."""
    import sys
    if "/opt/trn_rl_repo" not in sys.path:
        sys.path.insert(0, "/opt/trn_rl_repo")
    import concourse.tile as tile
    import concourse.mybir as mybir
    from concourse import bacc

    bf16 = mybir.dt.bfloat16
    f32 = mybir.dt.float32
    AFT = mybir.ActivationFunctionType

    TBS = 512
    NTBS = T // TBS      # 4
    NTS = TBS // P       # 4

    G = FC // fcg        # w0 DMA groups
    FW = fcg * P         # f elements per group (512)

    nc = bacc.Bacc(
        "TRN2",
        target_bir_lowering=False,
        debug=False,
        enable_asserts=True,
        num_devices=N_CORES,
    )

    # DRAM layouts are grouped so every load has long (8KB) contiguous
    # per-partition runs: xT by token-block, w0T by f-group.
    xT = nc.dram_tensor("xT", [P, NTBS, DC, TBS], bf16, kind="ExternalInput").ap()
    w0T = nc.dram_tensor("w0T", [P, G, DC, FW], bf16, kind="ExternalInput").ap()
    w3T = nc.dram_tensor("w3T", [P, FC, D], bf16, kind="ExternalInput").ap()
    out = nc.dram_tensor("out", [T, D], f32, kind="ExternalOutput").ap()

    with tile.TileContext(nc) as tc:
        with (
            tc.tile_pool(name="weights", bufs=1) as wpool,
            tc.tile_pool(name="xin", bufs=x_bufs) as xpool,
            tc.tile_pool(name="gelu", bufs=FC + g_extra) as gpool,
            tc.tile_pool(name="ostage", bufs=o_sb_bufs) as opool,
            tc.tile_pool(name="hps", bufs=h_bufs, space="PSUM") as hpsum,
            tc.tile_pool(name="ops", bufs=o_ps_bufs, space="PSUM") as opsum,
        ):
            # w0 SBUF mirrors the grouped DRAM layout; GEMM1 slices
            # [:, fc//fcg, dc, (fc%fcg)*P : +P].
            w0_sb = wpool.tile([P, G, DC, FW], bf16, name="w0_sb", tag="w0_sb")
            w3_sb = wpool.tile([P, FC, D], bf16, name="w3_sb", tag="w3_sb")

            x_tiles = {}
            def load_x(tb):
                xt = xpool.tile([P, DC, TBS], bf16, name=f"x_{tb}", tag="x")
                if x_coarse:
                    nc.sync.dma_start(xt[:], xT[:, tb])
                else:
                    for dc in range(DC):
                        nc.sync.dma_start(xt[:, dc], xT[:, tb, dc])
                x_tiles[tb] = xt

            if warmup_mms:
                with (
                    tc.tile_pool(name="warm", bufs=1) as warmpool,
                    tc.tile_pool(name="warmps", bufs=1, space="PSUM") as warmpsum,
                ):
                    wsrc = warmpool.tile([P, DW], bf16, name="wsrc", tag="wsrc")
                    wps = warmpsum.tile([P, DW], f32, name="wps", tag="wps")
                    nc.gpsimd.memset(wsrc[:], 0.0)
                    for i in range(warmup_mms):
                        nc.tensor.matmul(wps[:], wsrc[:, :P], wsrc[:],
                                         start=(i == 0), stop=(i == warmup_mms - 1))

            # critical prefix: x[tb0], then w0/w3 by f-group in consumption order
            load_x(0)
            for g in range(G):
                if w0_coarse:
                    nc.sync.dma_start(w0_sb[:, g], w0T[:, g])
                else:
                    for dc in range(DC):
                        nc.sync.dma_start(w0_sb[:, g, dc], w0T[:, g, dc])
                nc.sync.dma_start(w3_sb[:, g * fcg:(g + 1) * fcg],
                                  w3T[:, g * fcg:(g + 1) * fcg])

            for tb in range(NTBS):
                if tb + 1 < NTBS:
                    load_x(tb + 1)
                xt = x_tiles.pop(tb)
                # phase A: GEMM1 + GELU for all fc
                g_tiles = []
                for fc in range(FC):
                    h_ps = hpsum.tile([P, TBS], f32, name=f"h_{tb}_{fc}", tag="h_ps")
                    for dc in range(DC):
                        j = fc % fcg
                        nc.tensor.matmul(
                            h_ps[:],
                            w0_sb[:, fc // fcg, dc, j * P:(j + 1) * P],
                            xt[:, dc],
                            start=(dc == 0),
                            stop=(dc == DC - 1),
                        )
                    g_sb = gpool.tile([P, TBS], bf16, name=f"g_{tb}_{fc}", tag="g")
                    nc.scalar.activation(g_sb[:], h_ps[:], AFT.Gelu)
                    g_tiles.append(g_sb)
                # phase B: GEMM2, one [t=128, d=512] accumulation group at a time
                for ts in range(NTS):
                    for dc2 in range(2):
                        o_ps = opsum.tile([P, DW], f32, name=f"o_{tb}_{ts}_{dc2}",
                                          tag="o_ps")
                        for fc in range(FC):
                            nc.tensor.matmul(
                                o_ps[:],
                                g_tiles[fc][:, ts * P:(ts + 1) * P],
                                w3_sb[:, fc, dc2 * DW:(dc2 + 1) * DW],
                                start=(fc == 0),
                                stop=(fc == FC - 1),
                            )
                        o_sb = opool.tile([P, DW], f32, name=f"os_{tb}_{ts}_{dc2}",
                                          tag="o_sb")
                        nc.vector.tensor_copy(o_sb[:], o_ps[:])
                        nc.sync.dma_start(
                            out[tb * TBS + ts * P: tb * TBS + (ts + 1) * P,
                                dc2 * DW:(dc2 + 1) * DW],
                            o_sb[:],
                        )

    nc.compile()
    return nc


def _build_nc_v3(
    g32_bufs=4,
    h_bufs=3,
    o_ps_bufs=3,
    o_sb_bufs=3,
    x_bufs=4,
    warmup_mms=38,
    x_res_pairs=4,     # d-pairs (of 4) that get the x_lo@w0_hi product
    g_res_fpairs=16,   # f-pairs (of 16) that get the g_lo@w3_hi product
    w3_res_fpairs=10,  # f-pairs (of 16) that get the g_hi@w3_lo product
):
    """fp8e4 DoubleRow variant: every operand is represented as two fp8
    digits (hi = fp8(v*s), lo = fp8(v*s - hi)) sharing one scale s per
    tensor, and each 256-wide contraction pair is computed with 3 DoubleRow
    products (hi*hi, lo*hi, hi*lo) at 0.5 cycles/row — 12 products per
    output tile vs bf16's 16 matmuls.

    Scales: x*8, w0*256 -> h_psum = h*2048, GELU applied with input scale
    1/2048; g digits unscaled (s=1); w3*256 -> out_psum = out*256, final
    copy applies 1/256.
    """
    import sys
    if "/opt/trn_rl_repo" not in sys.path:
        sys.path.insert(0, "/opt/trn_rl_repo")
    import concourse.tile as tile
    import concourse.mybir as mybir
    from concourse import bacc

    fp8 = mybir.dt.float8e4
    f32 = mybir.dt.float32
    AFT = mybir.ActivationFunctionType
    DR = mybir.MatmulPerfMode.DoubleRow
    SUB = mybir.AluOpType.subtract

    TBS = 512
    NTBS = T // TBS      # 4
    NTS = TBS // P       # 4
    DPAIR = D // 256     # 4  (256-wide contraction pairs for GEMM1)
    FPAIR = F // 256     # 16 (for GEMM2)
    G1 = 16              # w0 DMA f-groups (small groups -> short critical prefix)
    FW = F // G1         # 256 f per group
    JG = FW // P         # 2 f-chunks per group

    nc = bacc.Bacc(
        "TRN2",
        target_bir_lowering=False,
        debug=False,
        enable_asserts=True,
        num_devices=N_CORES,
    )

    xh_d = nc.dram_tensor("xh", [P, NTBS, DPAIR, 2, TBS], fp8, kind="ExternalInput").ap()
    xl_d = nc.dram_tensor("xl", [P, NTBS, DPAIR, 2, TBS], fp8, kind="ExternalInput").ap()
    w0h_d = nc.dram_tensor("w0h", [P, G1, DPAIR, 2, FW], fp8, kind="ExternalInput").ap()
    w0l_d = nc.dram_tensor("w0l", [P, G1, DPAIR, 2, FW], fp8, kind="ExternalInput").ap()
    w3h_d = nc.dram_tensor("w3h", [P, FPAIR, 2, D], fp8, kind="ExternalInput").ap()
    w3l_d = nc.dram_tensor("w3l", [P, FPAIR, 2, D], fp8, kind="ExternalInput").ap()
    out = nc.dram_tensor("out", [T, D], f32, kind="ExternalOutput").ap()

    with tile.TileContext(nc) as tc:
        with (
            tc.tile_pool(name="weights", bufs=1) as wpool,
            tc.tile_pool(name="xin", bufs=x_bufs) as xpool,
            tc.tile_pool(name="g32", bufs=g32_bufs) as gpool,
            tc.tile_pool(name="ostage", bufs=o_sb_bufs) as opool,
            tc.tile_pool(name="hps", bufs=h_bufs, space="PSUM") as hpsum,
            tc.tile_pool(name="ops", bufs=o_ps_bufs, space="PSUM") as opsum,
        ):
            w0_sb = [wpool.tile([P, G1, DPAIR, 2, FW], fp8, name=f"w0{d}_sb",
                                tag=f"w0{d}") for d in "hl"]
            w3_sb = [wpool.tile([P, FPAIR, 2, D], fp8, name=f"w3{d}_sb",
                                tag=f"w3{d}") for d in "hl"]
            g_sb = [wpool.tile([P, FC, TBS], fp8, name=f"g{d}_sb",
                               tag=f"g{d}") for d in "hl"]

            x_tiles = {}
            def load_x(tb, digits=(0, 1)):
                xt = x_tiles.setdefault(tb, [None, None])
                for dgt in digits:
                    t = xpool.tile([P, DPAIR, 2, TBS], fp8,
                                   name=f"x{'hl'[dgt]}_{tb}", tag=f"x{'hl'[dgt]}")
                    nc.sync.dma_start(t[:], (xh_d, xl_d)[dgt][:, tb])
                    xt[dgt] = t

            if warmup_mms:
                with (
                    tc.tile_pool(name="warm", bufs=1) as warmpool,
                    tc.tile_pool(name="warmps", bufs=1, space="PSUM") as warmpsum,
                ):
                    wsrc = warmpool.tile([P, P], mybir.dt.bfloat16, name="wsrc",
                                         tag="wsrc")
                    wps = warmpsum.tile([P, P], f32, name="wps", tag="wps")
                    nc.vector.memset(wsrc[:], 0.0)
                    for i in range(warmup_mms):
                        nc.tensor.matmul(wps[:], wsrc[:], wsrc[:],
                                         start=(i == 0), stop=(i == warmup_mms - 1))

            # critical prefix in first-consumption order (products within an
            # fc run digit-major hh, lh, hl): xh(tb0), w0h g0, xl, w0l g0,
            # then remaining w0 groups, then w3 groups + remaining x blocks.
            load_x(0, digits=(0,))
            nc.sync.dma_start(w0_sb[0][:, 0], w0h_d[:, 0])
            load_x(0, digits=(1,))
            nc.sync.dma_start(w0_sb[1][:, 0], w0l_d[:, 0])
            # remaining w0 groups back-to-back (phase A consumes 1 group per
            # ~2.5us vs ~1.5us DMA), then all of w3 in fpair order (needed
            # from phase B(tb0) at ~48us; lands by ~50... the last fpairs are
            # read last), then the remaining x blocks (needed at ~90us+).
            for g in range(1, G1):
                nc.sync.dma_start(w0_sb[0][:, g], w0h_d[:, g])
                nc.sync.dma_start(w0_sb[1][:, g], w0l_d[:, g])
            for fp in range(FPAIR):
                nc.sync.dma_start(w3_sb[0][:, fp:fp + 1], w3h_d[:, fp:fp + 1])
                nc.sync.dma_start(w3_sb[1][:, fp:fp + 1], w3l_d[:, fp:fp + 1])
            for tb in range(1, NTBS):
                load_x(tb)

            for tb in range(NTBS):
                xt = x_tiles.pop(tb)
                # phase A: GEMM1 (DoubleRow, 12 products per fc) + GELU + digits
                for fc in range(FC):
                    g1, j = fc // JG, fc % JG
                    h_ps = hpsum.tile([P, TBS], f32, name=f"h_{tb}_{fc}", tag="h_ps")
                    prods = ([(p, 0, 0) for p in range(DPAIR)]
                             + [(p, 1, 0) for p in range(x_res_pairs)]
                             + [(p, 0, 1) for p in range(DPAIR)])
                    for i, (pair, xa, wb) in enumerate(prods):
                        nc.tensor.matmul(
                            h_ps[:],
                            w0_sb[wb][:, g1, pair, :, j * P:(j + 1) * P],
                            xt[xa][:, pair],
                            start=(i == 0),
                            stop=(i == len(prods) - 1),
                            perf_mode=DR,
                        )
                    g32 = gpool.tile([P, TBS], f32, name=f"g32_{tb}_{fc}", tag="g32")
                    nc.scalar.activation(g32[:], h_ps[:], AFT.Gelu,
                                         scale=1.0 / 2048.0)
                    nc.vector.tensor_copy(g_sb[0][:, fc], g32[:])
                    nc.vector.tensor_tensor(g_sb[1][:, fc], g32[:],
                                            g_sb[0][:, fc], SUB)
                # phase B: GEMM2 (DoubleRow over 16 f-pairs x 3 products)
                for ts in range(NTS):
                    for dc2 in range(2):
                        o_ps = opsum.tile([P, DW], f32, name=f"o_{tb}_{ts}_{dc2}",
                                          tag="o_ps")
                        prods = []
                        for fp in range(FPAIR):
                            prods.append((fp, 0, 0))
                            if fp < g_res_fpairs:
                                prods.append((fp, 1, 0))
                            if fp < w3_res_fpairs:
                                prods.append((fp, 0, 1))
                        for i, (fp, ga, wb) in enumerate(prods):
                            nc.tensor.matmul(
                                o_ps[:],
                                g_sb[ga][:, 2 * fp:2 * fp + 2,
                                         ts * P:(ts + 1) * P],
                                w3_sb[wb][:, fp, :, dc2 * DW:(dc2 + 1) * DW],
                                start=(i == 0),
                                stop=(i == len(prods) - 1),
                                perf_mode=DR,
                            )
                        o_sb = opool.tile([P, DW], f32, name=f"os_{tb}_{ts}_{dc2}",
                                          tag="o_sb")
                        nc.scalar.activation(o_sb[:], o_ps[:], AFT.Copy,
                                             scale=1.0 / 256.0)
                        nc.sync.dma_start(
                            out[tb * TBS + ts * P: tb * TBS + (ts + 1) * P,
                                dc2 * DW:(dc2 + 1) * DW],
                            o_sb[:],
                        )

    nc.compile()
    return nc


def _get_nc():
    if "nc" not in _cache:
        _cache["nc"] = _build_nc_v3()
    return _cache["nc"]


def _make_cached_fn(nc):
    """Build a reusable jitted 8-core executable around bass2jax's bass_exec
    primitive (the same lowering run_bass_kernel_spmd uses under axon), so
    repeat kernel() calls skip retrace/relower."""
    import jax
    import numpy as np
    from jax.sharding import Mesh, PartitionSpec
    try:
        from jax.experimental.shard_map import shard_map
    except ImportError:
        from jax.shard_map import shard_map
    import concourse.mybir as mybir
    from concourse.bass2jax import (_bass_exec_p, install_neuronx_cc_hook,
                                    partition_id_tensor)

    install_neuronx_cc_hook()
    partition_name = nc.partition_id_tensor.name if nc.partition_id_tensor else None
    in_names, out_names, out_avals, zero_shapes = [], [], [], []
    for alloc in nc.m.functions[0].allocations:
        if not isinstance(alloc, mybir.MemoryLocationSet):
            continue
        name = alloc.memorylocations[0].name
        if alloc.kind == "ExternalInput":
            if name != partition_name:
                in_names.append(name)
        elif alloc.kind == "ExternalOutput":
            out_names.append(name)
            shape = tuple(alloc.tensor_shape)
            dtype = mybir.dt.np(alloc.dtype)
            out_avals.append(jax.core.ShapedArray(shape, dtype))
            zero_shapes.append((shape, dtype))
    n_params = len(in_names)
    all_in_names = list(in_names) + list(out_names)
    if partition_name is not None:
        all_in_names.append(partition_name)

    def _body(*args):
        ins = list(args[:n_params])
        outs = list(args[n_params:])
        extra = [partition_id_tensor()] if partition_name is not None else []
        return tuple(_bass_exec_p.bind(
            *ins, *outs, *extra,
            out_avals=tuple(out_avals),
            in_names=tuple(all_in_names),
            out_names=tuple(out_names),
            lowering_input_output_aliases=(),
            sim_require_finite=True,
            sim_require_nnan=True,
            nc=nc,
        ))

    devices = jax.devices()[:N_CORES]
    mesh = Mesh(np.asarray(devices), ("core",))
    fn = jax.jit(
        shard_map(_body, mesh=mesh,
                  in_specs=(PartitionSpec("core"),) * (n_params + len(out_names)),
                  out_specs=(PartitionSpec("core"),) * len(out_names),
                  check_rep=False),
        keep_unused=True)

    def run(in_maps):
        concat_in = [np.concatenate([np.asarray(m[n]) for m in in_maps], axis=0)
                     for n in in_names]
        concat_zeros = [np.zeros((N_CORES * s[0], *s[1:]), dt)
                        for s, dt in zero_shapes]
        outs = fn(*concat_in, *concat_zeros)
        return [
            {name: np.asarray(outs[i]).reshape(N_CORES, *out_avals[i].shape)[c]
             for i, name in enumerate(out_names)}
            for c in range(N_CORES)
        ]

    return run


def kernel(**inputs):
    import os
    import sys
    if "/opt/trn_rl_repo" not in sys.path:
        sys.path.insert(0, "/opt/trn_rl_repo")
    from concourse import bass_utils

    output_tensor = np.asarray(inputs["output_tensor"], dtype=np.float32)  # [1, 8]
    x = np.asarray(inputs["inputs"], dtype=np.float32)   # [1, 8, 2048, 1024]
    w0 = np.asarray(inputs["w0"], dtype=np.float32)      # [8, 4096, 1024]
    w3 = np.asarray(inputs["w3"], dtype=np.float32)      # [8, 1024, 4096]

    fp8 = ml_dtypes.float8_e4m3
    TBS, NTBS = 512, T // 512
    DPAIR, FPAIR = D // 256, F // 256
    G1 = 16
    FW = F // G1

    def split2(a):
        """hi = fp8(a), lo = fp8(a - hi): 2-digit fp8 representation."""
        hi = a.astype(fp8)
        lo = (a - hi.astype(np.float32)).astype(fp8)
        return hi, lo

    def prep_expert(e):
        # Contraction-major layouts with the 256-wide DoubleRow pair split:
        # contraction index c = pair*256 + slot*128 + ki (ki = partition).
        #   x  [128 ki, 4 tb, 4 pair, 2 slot, 512 t]   (scale 8)
        #   w0 [128 ki, 8 g, 4 pair, 2 slot, 512 fw]   (scale 256)
        #   w3 [128 ki, 16 fp, 2 slot, 1024 d]         (scale 256)
        xh, xl = split2(x[0, e].T * 8.0)          # [D, T]
        w0h, w0l = split2(w0[e].T * 256.0)        # [D, F]
        w3h, w3l = split2(w3[e].T * 256.0)        # [F, D]

        def xlay(a):
            return np.ascontiguousarray(
                a.reshape(DPAIR, 2, P, NTBS, TBS).transpose(2, 3, 0, 1, 4))

        def w0lay(a):
            return np.ascontiguousarray(
                a.reshape(DPAIR, 2, P, G1, FW).transpose(2, 3, 0, 1, 4))

        def w3lay(a):
            return np.ascontiguousarray(
                a.reshape(FPAIR, 2, P, D).transpose(2, 0, 1, 3))

        return {
            "xh": xlay(xh), "xl": xlay(xl),
            "w0h": w0lay(w0h), "w0l": w0lay(w0l),
            "w3h": w3lay(w3h), "w3l": w3lay(w3l),
        }

    from concurrent.futures import ThreadPoolExecutor
    with ThreadPoolExecutor(max_workers=N_CORES) as pool:
        in_maps = list(pool.map(prep_expert, range(N_CORES)))

    nc = _get_nc()
    results = None
    if "fast_fn" in _cache:
        try:
            results = _cache["fast_fn"](in_maps)
        except Exception:
            results = None
    if results is None:
        try:
            results = bass_utils.run_bass_kernel_spmd(
                nc, in_maps, core_ids=list(range(N_CORES))).results
        except ModuleNotFoundError:
            # trace path requested via env but axon NTFF hook missing
            os.environ["BASS_NEVER_TRACE"] = "1"
            results = bass_utils.run_bass_kernel_spmd(
                nc, in_maps, core_ids=list(range(N_CORES))).results
        try:
            fast = _make_cached_fn(nc)
            fast(in_maps)  # warm: jit trace + XLA/NEFF compile happens here
            _cache["fast_fn"] = fast
        except Exception:
            pass
    out_full = np.stack([results[e]["out"] for e in range(N_CORES)])[None]

    # unpopular experts with zero gating activity produce zeros
    unpop = output_tensor[:, NUM_LOCAL:].sum(axis=0) != 0
    mask = np.concatenate([np.ones(NUM_LOCAL, dtype=bool), unpop])
    out_full = out_full * mask[None, :, None, None].astype(np.float32)
    return out_full.astype(np.float32)

